# revision 15
# baseline (speedup 1.0000x reference)
"""Bass/Trainium2 kernel for binarized AlexNet-OWT-BN (MNIST-shaped), 8-core data parallel.

Contract: kernel(**inputs) takes the FULL unsharded inputs (x: [8192,1,28,28] f32
plus conv/bn/linear params) and returns the FULL [8192,10] f32 log-softmax output.

Design notes
------------
- Data parallel: batch 8192 -> 8 cores x 1024 images; each core runs 2 blocks of
  T=512 images (512 = fp32 PSUM bank limit for the matmul moving dim).
- All convs are binarized: layer-1 input is sign(x) in {-1,+1}; inner activations
  are sign(relu(bn(.))) which equals indicator(bn>0) in {0,1}. We keep inner
  activations in the {-1,+1} "tilde" domain (a~ = 2a-1, pad slots = -1) so a
  single ScalarE Sign pass implements BN+ReLU+binarize; conv weights are halved
  and the constant 0.5*sum(w) folds into the per-channel threshold bias.
- BN channels with negative scale g*rsqrt(v+eps) are handled on the host by
  negating that output channel's weights and threshold (makes max-pooling
  universally correct in the tilde domain).
- Conv as matmul: partitions hold one padded image row (padded_ix x ci <= 128),
  M = out_width x co <= 128 (block-Toeplitz weights built on host), free dim =
  images; the 3 ky taps are PSUM-accumulated matmuls whose rhs differs only by a
  free-dim offset (rows live in the free dim: (padded_row, image)).
- 2x2 maxpool: the Sign threshold runs per conv row straight from PSUM (PSUM
  allows only one tensor_tensor operand, so pooling happens on the binarized
  SBUF values where ops run in fast 2x bf16 mode); y-pairs max lane-aligned,
  x-pairs use a parity-major M layout (even px at partitions 0-55, odd px at
  64-119) so one quadrant-aligned copy plus one aligned max pools in-lane.
- conv1/conv3 use 4x/2x TensorE row tiling (tile_position row bands, inputs
  replicated across SBUF quadrants by cheap DVE copies) since K=30/64 << 128.
- Head: 7 accumulated matmuls against a channel-summing matrix implement the
  7x7 mean; Sign gives the binarized features; a 16->10 matmul, TensorE
  transpose per 128-image chunk, and a free-dim log-softmax finish.
"""

import sys

sys.path.insert(0, "/opt/trn_rl_repo")

import numpy as np
import ml_dtypes

EPS = 1e-5
T = 512
NBLK = 2
NCORE = 8
NPER = T * NBLK  # images per core

BF16 = ml_dtypes.bfloat16

# layer geometry: (ci, co, W_out, pooled)
LAYERS = {
    1: dict(ci=1, co=4, W=28),
    2: dict(ci=4, co=4, W=28),
    3: dict(ci=4, co=8, W=14),
    4: dict(ci=8, co=8, W=14),
    5: dict(ci=8, co=16, W=7),
}


def _krow(ix, c, W, ci):
    """K-partition index for padded column ix: real px first, pads at the end."""
    if ix == 0:
        return W * ci + c
    if ix == W + 1:
        return W * ci + ci + c
    return (ix - 1) * ci + c


def _mcol(ox, c, W, co, parity):
    """M column for out px ox; parity-major (evens/gap/odds) for pool layers."""
    if not parity:
        return ox * co + c
    half = (W // 2) * co
    pad_half = ((half + 31) // 32) * 32  # odds start at the next quadrant
    if ox % 2 == 0:
        return (ox // 2) * co + c
    return pad_half + (ox // 2) * co + c


def _mwidth(W, co, parity):
    if not parity:
        return W * co
    half = (W // 2) * co
    pad_half = ((half + 31) // 32) * 32
    return pad_half + half


def _toeplitz(wmat, W, parity):
    co, ci = wmat.shape[0], wmat.shape[1]
    K = (W + 2) * ci
    M = _mwidth(W, co, parity)
    out = []
    for ky in range(3):
        Wk = np.zeros((K, M), np.float32)
        for ox in range(W):
            for kx in range(3):
                ix = ox + kx
                for c_o in range(co):
                    for c_i in range(ci):
                        Wk[_krow(ix, c_i, W, ci),
                           _mcol(ox, c_o, W, co, parity)] = wmat[c_o, c_i, ky, kx]
        out.append(Wk)
    return np.stack(out)  # [3, K, M]


def _host_fold(inputs):
    """Fold weights + BN into Toeplitz matmul weights and threshold vectors."""
    d = {}
    for l in range(1, 6):
        tag = str(l)
        w = np.asarray(inputs["w" + tag], np.float64)
        b = np.asarray(inputs["b" + tag], np.float64)
        g = np.asarray(inputs["g" + tag], np.float64)
        be = np.asarray(inputs["be" + tag], np.float64)
        m = np.asarray(inputs["m" + tag], np.float64)
        v = np.asarray(inputs["v" + tag], np.float64)
        wb = np.sign(w).astype(np.float64)
        s = g / np.sqrt(v + EPS)
        geo = LAYERS[l]
        co, W = geo["co"], geo["W"]
        if l <= 4:
            # threshold: bn(y_conv) > 0  <=>  psum + bias > 0 (after folding)
            c = (b - m) + be / s
            flip = np.where(s < 0, -1.0, 1.0)
            wb = wb * flip[:, None, None, None]
            c = c * flip
            if l == 1:
                wmat, kap = wb, np.zeros(co)
            else:
                wmat = wb * 0.5
                kap = 0.5 * wb.sum(axis=(1, 2, 3))
            parity = l in (2, 4)
            bias_ch = (kap + c).astype(np.float32)  # per channel, > 0 test
            M = _mwidth(W, co, parity)
            bias_vec = np.zeros((M, 1), np.float32)
            for ox in range(W):
                for c_o in range(co):
                    bias_vec[_mcol(ox, c_o, W, co, parity), 0] = bias_ch[c_o]
            d[f"thr{l}"] = bias_vec
            d[f"wk{l}"] = _toeplitz(wmat.astype(np.float32), W, parity).astype(BF16)
        else:
            wmat = wb * 0.5
            kap = 0.5 * wb.sum(axis=(1, 2, 3))
            d["wk5"] = _toeplitz(wmat.astype(np.float32), W, False).astype(BF16)
            # head mean+bn5: z = (s5/49)*psum_sum + s5*(kap+b5-m5)+be5
            d["s5"] = (s / 49.0).astype(np.float32).reshape(-1, 1)
            d["b5"] = (s * (kap + b - m) + be).astype(np.float32).reshape(-1, 1)
    # channel-summing matrix for the 7x7 mean: [112=(ox,co), 16]
    S = np.zeros((7 * 16, 16), np.float32)
    for ox in range(7):
        for c_ in range(16):
            S[ox * 16 + c_, c_] = 1.0
    d["Ssum"] = S.astype(BF16)
    wl = np.sign(np.asarray(inputs["wl"], np.float64))  # [10, 16]
    bl = np.asarray(inputs["bl"], np.float64)
    d["whead"] = (wl.T * 0.5).astype(BF16)  # [16, 10] lhsT
    d["bhead"] = (bl + 0.5 * wl.sum(axis=1)).astype(np.float32).reshape(-1, 1)
    d["id10"] = np.eye(10, dtype=np.float32)
    return d


_CACHE = {}


def _build():
    from concourse import bacc, tile, mybir

    f32 = mybir.dt.float32
    bf16 = mybir.dt.bfloat16
    ACT = mybir.ActivationFunctionType
    ALU = mybir.AluOpType
    AX = mybir.AxisListType

    nc = bacc.Bacc("TRN2", num_devices=NCORE)

    xT = nc.dram_tensor("xT", (784, NPER), f32, kind="ExternalInput")
    dr = {}
    for l in range(1, 6):
        geo = LAYERS[l]
        K = (geo["W"] + 2) * geo["ci"]
        M = _mwidth(geo["W"], geo["co"], l in (2, 4))
        dr[f"wk{l}"] = nc.dram_tensor(f"wk{l}", (3, K, M), bf16, kind="ExternalInput")
        if l <= 4:
            dr[f"thr{l}"] = nc.dram_tensor(f"thr{l}", (M, 1), f32, kind="ExternalInput")
    dr["Ssum"] = nc.dram_tensor("Ssum", (112, 16), bf16, kind="ExternalInput")
    dr["whead"] = nc.dram_tensor("whead", (16, 10), bf16, kind="ExternalInput")
    dr["s5"] = nc.dram_tensor("s5", (16, 1), f32, kind="ExternalInput")
    dr["b5"] = nc.dram_tensor("b5", (16, 1), f32, kind="ExternalInput")
    dr["bhead"] = nc.dram_tensor("bhead", (10, 1), f32, kind="ExternalInput")
    dr["id10"] = nc.dram_tensor("id10", (10, 10), f32, kind="ExternalInput")
    out = nc.dram_tensor("out", (NPER, 10), f32, kind="ExternalOutput")

    with tile.TileContext(nc) as tc:
        stat = tc.alloc_tile_pool(name="stat", bufs=1)
        scr = tc.alloc_tile_pool(name="scr", bufs=4)
        ps = tc.alloc_tile_pool(name="ps", bufs=4, space="PSUM")
        ps_pm = tc.alloc_tile_pool(name="ps_pm", bufs=1, space="PSUM")
        ps_ph = tc.alloc_tile_pool(name="ps_ph", bufs=1, space="PSUM")
        ps_tr = tc.alloc_tile_pool(name="ps_tr", bufs=2, space="PSUM")

        # --- static buffers ---
        xraw = stat.tile([128, 30 * T], f32, tag="xraw")
        xb = stat.tile([128, 30 * T], bf16, tag="xb")
        a2 = stat.tile([128, 30 * T], bf16, tag="a2")
        a3 = stat.tile([128, 16 * T], bf16, tag="a3")
        a4 = stat.tile([128, 16 * T], bf16, tag="a4")
        a5b = stat.tile([128, 9 * T], bf16, tag="a5b")
        y5 = stat.tile([128, 7 * T], bf16, tag="y5")

        wt = {}
        for l in range(1, 6):
            geo = LAYERS[l]
            K = (geo["W"] + 2) * geo["ci"]
            M = _mwidth(geo["W"], geo["co"], l in (2, 4))
            for ky in range(3):
                t = stat.tile([128, 128], bf16, tag=f"w{l}_{ky}")
                if l == 1:
                    for b_ in range(4):
                        nc.sync.dma_start(t[32 * b_:32 * b_ + K, 0:M],
                                          dr[f"wk{l}"].ap()[ky, :, :])
                elif l == 3:
                    for b_ in range(2):
                        nc.sync.dma_start(t[64 * b_:64 * b_ + K, 0:M],
                                          dr[f"wk{l}"].ap()[ky, :, :])
                else:
                    nc.sync.dma_start(t[0:K, 0:M], dr[f"wk{l}"].ap()[ky, :, :])
                wt[(l, ky)] = t
        Ssb = stat.tile([128, 16], bf16, tag="Ssb")
        nc.sync.dma_start(Ssb[0:112, 0:16], dr["Ssum"].ap())
        whd = stat.tile([16, 16], bf16, tag="whd")
        nc.sync.dma_start(whd[0:16, 0:10], dr["whead"].ap())
        id10 = stat.tile([10, 16], f32, tag="id10")
        nc.sync.dma_start(id10[0:10, 0:10], dr["id10"].ap())
        cvec = {}
        for name, P in [("thr1", 112), ("thr2", 120), ("thr3", 112),
                        ("thr4", 120), ("s5", 16), ("b5", 16), ("bhead", 10)]:
            t = stat.tile([128, 1], f32, tag="c_" + name)
            nc.sync.dma_start(t[0:P, 0:1], dr[name].ap())
            cvec[name] = t

        # --- init: zero x pads, set tilde-domain buffers (pads) to -1 ---
        nc.scalar.memzero(xraw[:, :])
        for buf, fp in [(a2, 30 * T), (a3, 16 * T), (a4, 16 * T), (a5b, 9 * T)]:
            nc.scalar.memzero(buf[:, 0:fp])
            nc.vector.tensor_scalar_add(buf[:, 0:fp], buf[:, 0:fp], -1.0)

        def conv_rows(l, src, nrows, psum_for_row):
            """Emit 3 accumulated matmuls per output row; returns list of psum tiles."""
            geo = LAYERS[l]
            K = (geo["W"] + 2) * geo["ci"]
            M = geo["W"] * geo["co"]
            pts = []
            for y in range(nrows):
                pt = psum_for_row()
                for ky in range(3):
                    nc.tensor.matmul(
                        out=pt[0:M, :],
                        lhsT=wt[(l, ky)][0:K, 0:M],
                        rhs=src[0:K, (y + ky) * T:(y + ky + 1) * T],
                        start=(ky == 0),
                        stop=(ky == 2),
                        tile_position=(0, 0),
                    )
                pts.append(pt)
            return pts

        for blk in range(NBLK):
            ioff = blk * T
            # ---- load + sign x ----
            src = xT.ap()[:, ioff:ioff + T].rearrange("(r c) n -> c r n", r=28)
            dst = xraw[0:28, T:29 * T].rearrange("p (r t) -> p r t", r=28)
            nc.sync.dma_start(dst, src)
            nc.scalar.activation(xb[0:30, :], xraw[0:30, :], ACT.Sign)
            for b_ in range(1, 4):
                nc.vector.tensor_copy(xb[32 * b_:32 * b_ + 30, :], xb[0:30, :])

            # ---- L1 ----
            for y in range(28):
                b_ = 32 * (y % 4)
                pt = ps.tile([128, T], f32, tag="pt")
                for ky in range(3):
                    nc.tensor.matmul(
                        out=pt[0:112, :], lhsT=wt[(1, ky)][b_:b_ + 30, 0:112],
                        rhs=xb[b_:b_ + 30, (y + ky) * T:(y + ky + 1) * T],
                        start=(ky == 0), stop=(ky == 2), tile_position=(b_, 0))
                nc.scalar.activation(
                    a2[0:112, (y + 1) * T:(y + 2) * T], pt[0:112, :],
                    ACT.Sign, bias=cvec["thr1"][0:112, 0:1])

            # ---- L2 (pool) ----
            for r in range(14):
                pt0 = ps.tile([128, T], f32, tag="pt")
                pt1 = ps.tile([128, T], f32, tag="pt")
                for (y, pt) in ((2 * r, pt0), (2 * r + 1, pt1)):
                    for ky in range(3):
                        nc.tensor.matmul(
                            out=pt[0:120, :], lhsT=wt[(2, ky)][0:120, 0:120],
                            rhs=a2[0:120, (y + ky) * T:(y + ky + 1) * T],
                            start=(ky == 0), stop=(ky == 2), tile_position=(0, 0))
                q0 = scr.tile([128, T], bf16, tag="q0")
                q1 = scr.tile([128, T], bf16, tag="q1")
                nc.scalar.activation(q0[0:120, :], pt0[0:120, :], ACT.Sign,
                                     bias=cvec["thr2"][0:120, 0:1])
                nc.scalar.activation(q1[0:120, :], pt1[0:120, :], ACT.Sign,
                                     bias=cvec["thr2"][0:120, 0:1])
                sy = scr.tile([128, T], bf16, tag="sy")
                nc.vector.tensor_tensor(out=sy[0:120, :], in0=q0[0:120, :],
                                        in1=q1[0:120, :], op=ALU.max)
                sq = scr.tile([128, T], bf16, tag="sq")
                nc.vector.tensor_copy(sq[0:56, :], sy[64:120, :])
                nc.vector.tensor_tensor(out=a3[0:56, (r + 1) * T:(r + 2) * T],
                                        in0=sy[0:56, :], in1=sq[0:56, :],
                                        op=ALU.max)

            # ---- L3 (2-way row tiling; replicate a3 to the upper half) ----
            nc.vector.tensor_copy(a3[64:128, :], a3[0:64, :])
            for y in range(14):
                b_ = 64 * (y % 2)
                pt = ps.tile([128, T], f32, tag="pt")
                for ky in range(3):
                    nc.tensor.matmul(
                        out=pt[0:112, :], lhsT=wt[(3, ky)][b_:b_ + 64, 0:112],
                        rhs=a3[b_:b_ + 64, (y + ky) * T:(y + ky + 1) * T],
                        start=(ky == 0), stop=(ky == 2), tile_position=(b_, 0))
                nc.scalar.activation(
                    a4[0:112, (y + 1) * T:(y + 2) * T], pt[0:112, :],
                    ACT.Sign, bias=cvec["thr3"][0:112, 0:1])

            # ---- L4 (pool) ----
            for r in range(7):
                pt0 = ps.tile([128, T], f32, tag="pt")
                pt1 = ps.tile([128, T], f32, tag="pt")
                for (y, pt) in ((2 * r, pt0), (2 * r + 1, pt1)):
                    for ky in range(3):
                        nc.tensor.matmul(
                            out=pt[0:120, :], lhsT=wt[(4, ky)][0:128, 0:120],
                            rhs=a4[0:128, (y + ky) * T:(y + ky + 1) * T],
                            start=(ky == 0), stop=(ky == 2), tile_position=(0, 0))
                q0 = scr.tile([128, T], bf16, tag="q0")
                q1 = scr.tile([128, T], bf16, tag="q1")
                nc.scalar.activation(q0[0:120, :], pt0[0:120, :], ACT.Sign,
                                     bias=cvec["thr4"][0:120, 0:1])
                nc.scalar.activation(q1[0:120, :], pt1[0:120, :], ACT.Sign,
                                     bias=cvec["thr4"][0:120, 0:1])
                sy = scr.tile([128, T], bf16, tag="sy")
                nc.vector.tensor_tensor(out=sy[0:120, :], in0=q0[0:120, :],
                                        in1=q1[0:120, :], op=ALU.max)
                sq = scr.tile([128, T], bf16, tag="sq")
                nc.vector.tensor_copy(sq[0:56, :], sy[64:120, :])
                nc.vector.tensor_tensor(out=a5b[0:56, (r + 1) * T:(r + 2) * T],
                                        in0=sy[0:56, :], in1=sq[0:56, :],
                                        op=ALU.max)

            # ---- L5 conv -> y5 (raw values, bf16-exact halves) ----
            for y in range(7):
                pt = ps.tile([128, T], f32, tag="pt")
                for ky in range(3):
                    nc.tensor.matmul(
                        out=pt[0:112, :], lhsT=wt[(5, ky)][0:72, 0:112],
                        rhs=a5b[0:72, (y + ky) * T:(y + ky + 1) * T],
                        start=(ky == 0), stop=(ky == 2), tile_position=(0, 0))
                nc.scalar.activation(y5[0:112, y * T:(y + 1) * T],
                                     pt[0:112, :], ACT.Copy)

            # ---- head: mean over 7x7 via 7 accumulated matmuls ----
            pm = ps_pm.tile([128, T], f32, tag="pm")
            for r in range(7):
                nc.tensor.matmul(
                    out=pm[0:16, :], lhsT=Ssb[0:112, 0:16],
                    rhs=y5[0:112, r * T:(r + 1) * T],
                    start=(r == 0), stop=(r == 6), tile_position=(0, 0))
            u = scr.tile([16, T], bf16, tag="u")
            nc.scalar.activation(u[0:16, :], pm[0:16, :], ACT.Sign,
                                 bias=cvec["b5"][0:16, 0:1],
                                 scale=cvec["s5"][0:16, 0:1])
            ph = ps_ph.tile([128, T], f32, tag="ph")
            nc.tensor.matmul(out=ph[0:10, :], lhsT=whd[0:16, 0:10],
                             rhs=u[0:16, :], start=True, stop=True,
                             tile_position=(0, 0))
            hh = scr.tile([16, T], f32, tag="hh")
            nc.scalar.activation(hh[0:10, :], ph[0:10, :], ACT.Identity,
                                 bias=cvec["bhead"][0:10, 0:1])

            for k in range(T // 128):
                ptr = ps_tr.tile([128, 16], f32, tag="ptr")
                nc.tensor.transpose(ptr[0:128, 0:10],
                                    hh[0:10, k * 128:(k + 1) * 128],
                                    id10[0:10, 0:10])
                mx = scr.tile([128, 1], f32, tag="mx")
                nc.vector.tensor_reduce(mx[0:128, 0:1], ptr[0:128, 0:10],
                                        axis=AX.X, op=ALU.max, negate=True)
                ex = scr.tile([128, 16], f32, tag="ex")
                nc.scalar.activation(ex[0:128, 0:10], ptr[0:128, 0:10],
                                     ACT.Exp, bias=mx[0:128, 0:1])
                sm = scr.tile([128, 1], f32, tag="sm")
                nc.vector.tensor_reduce(sm[0:128, 0:1], ex[0:128, 0:10],
                                        axis=AX.X, op=ALU.add)
                lg = scr.tile([128, 1], f32, tag="lg")
                nc.scalar.activation(lg[0:128, 0:1], sm[0:128, 0:1], ACT.Ln)
                tt = scr.tile([128, 1], f32, tag="tt")
                nc.vector.tensor_tensor(out=tt[0:128, 0:1], in0=mx[0:128, 0:1],
                                        in1=lg[0:128, 0:1], op=ALU.subtract)
                osb = scr.tile([128, 16], f32, tag="osb")
                nc.scalar.activation(osb[0:128, 0:10], ptr[0:128, 0:10],
                                     ACT.Identity, bias=tt[0:128, 0:1])
                row0 = ioff + k * 128
                nc.sync.dma_start(out.ap()[row0:row0 + 128, 0:10],
                                  osb[0:128, 0:10])

        for p in (ps_tr, ps_ph, ps_pm, ps, scr, stat):
            p.release()

    nc.compile()
    return nc


def kernel(**inputs):
    from concourse.bass_utils import run_bass_kernel_spmd
    import os

    if "nc" not in _CACHE:
        _CACHE["nc"] = _build()
    nc = _CACHE["nc"]

    folded = _host_fold(inputs)
    x = np.asarray(inputs["x"], np.float32).reshape(8192, 784)
    xT_full = np.ascontiguousarray(x.T)  # [784, 8192]

    in_maps = []
    for i in range(NCORE):
        m = {"xT": np.ascontiguousarray(xT_full[:, i * NPER:(i + 1) * NPER])}
        for k, v in folded.items():
            m[k] = v
        in_maps.append(m)

    res = run_bass_kernel_spmd(nc, in_maps, core_ids=list(range(NCORE)))
    _CACHE["last_result"] = res
    outs = [res.results[i]["out"] for i in range(NCORE)]
    return np.concatenate(outs, axis=0).astype(np.float32)


# revision 16
# speedup vs baseline: 1.0060x; 1.0060x over previous
"""Bass/Trainium2 kernel for binarized AlexNet-OWT-BN (MNIST-shaped), 8-core data parallel.

Contract: kernel(**inputs) takes the FULL unsharded inputs (x: [8192,1,28,28] f32
plus conv/bn/linear params) and returns the FULL [8192,10] f32 log-softmax output.

Design notes
------------
- Data parallel: batch 8192 -> 8 cores x 1024 images; each core runs 2 blocks of
  T=512 images (512 = fp32 PSUM bank limit for the matmul moving dim).
- All convs are binarized: layer-1 input is sign(x) in {-1,+1}; inner activations
  are sign(relu(bn(.))) which equals indicator(bn>0) in {0,1}. We keep inner
  activations in the {-1,+1} "tilde" domain (a~ = 2a-1, pad slots = -1) so a
  single ScalarE Sign pass implements BN+ReLU+binarize; conv weights are halved
  and the constant 0.5*sum(w) folds into the per-channel threshold bias.
- BN channels with negative scale g*rsqrt(v+eps) are handled on the host by
  negating that output channel's weights and threshold (makes max-pooling
  universally correct in the tilde domain).
- Conv as matmul: partitions hold one padded image row (padded_ix x ci <= 128),
  M = out_width x co <= 128 (block-Toeplitz weights built on host), free dim =
  images; the 3 ky taps are PSUM-accumulated matmuls whose rhs differs only by a
  free-dim offset (rows live in the free dim: (padded_row, image)).
- 2x2 maxpool: the Sign threshold runs per conv row straight from PSUM (PSUM
  allows only one tensor_tensor operand, so pooling happens on the binarized
  SBUF values where ops run in fast 2x bf16 mode); y-pairs max lane-aligned,
  x-pairs use a parity-major M layout (even px at partitions 0-55, odd px at
  64-119) so one quadrant-aligned copy plus one aligned max pools in-lane.
- conv1/conv3 use 4x/2x TensorE row tiling (tile_position row bands, inputs
  replicated across SBUF quadrants by cheap DVE copies) since K=30/64 << 128.
- Head: 7 accumulated matmuls against a channel-summing matrix implement the
  7x7 mean; Sign gives the binarized features; a 16->10 matmul, TensorE
  transpose per 128-image chunk, and a free-dim log-softmax finish.
"""

import sys

sys.path.insert(0, "/opt/trn_rl_repo")

import numpy as np
import ml_dtypes

EPS = 1e-5
T = 512
NBLK = 2
NCORE = 8
NPER = T * NBLK  # images per core

BF16 = ml_dtypes.bfloat16

# layer geometry: (ci, co, W_out, pooled)
LAYERS = {
    1: dict(ci=1, co=4, W=28),
    2: dict(ci=4, co=4, W=28),
    3: dict(ci=4, co=8, W=14),
    4: dict(ci=8, co=8, W=14),
    5: dict(ci=8, co=16, W=7),
}


def _krow(ix, c, W, ci):
    """K-partition index for padded column ix: real px first, pads at the end."""
    if ix == 0:
        return W * ci + c
    if ix == W + 1:
        return W * ci + ci + c
    return (ix - 1) * ci + c


def _mcol(ox, c, W, co, parity):
    """M column for out px ox; parity-major (evens/gap/odds) for pool layers."""
    if not parity:
        return ox * co + c
    half = (W // 2) * co
    pad_half = ((half + 31) // 32) * 32  # odds start at the next quadrant
    if ox % 2 == 0:
        return (ox // 2) * co + c
    return pad_half + (ox // 2) * co + c


def _mwidth(W, co, parity):
    if not parity:
        return W * co
    half = (W // 2) * co
    pad_half = ((half + 31) // 32) * 32
    return pad_half + half


def _toeplitz(wmat, W, parity):
    co, ci = wmat.shape[0], wmat.shape[1]
    K = (W + 2) * ci
    M = _mwidth(W, co, parity)
    out = []
    for ky in range(3):
        Wk = np.zeros((K, M), np.float32)
        for ox in range(W):
            for kx in range(3):
                ix = ox + kx
                for c_o in range(co):
                    for c_i in range(ci):
                        Wk[_krow(ix, c_i, W, ci),
                           _mcol(ox, c_o, W, co, parity)] = wmat[c_o, c_i, ky, kx]
        out.append(Wk)
    return np.stack(out)  # [3, K, M]


def _host_fold(inputs):
    """Fold weights + BN into Toeplitz matmul weights and threshold vectors."""
    d = {}
    for l in range(1, 6):
        tag = str(l)
        w = np.asarray(inputs["w" + tag], np.float64)
        b = np.asarray(inputs["b" + tag], np.float64)
        g = np.asarray(inputs["g" + tag], np.float64)
        be = np.asarray(inputs["be" + tag], np.float64)
        m = np.asarray(inputs["m" + tag], np.float64)
        v = np.asarray(inputs["v" + tag], np.float64)
        wb = np.sign(w).astype(np.float64)
        s = g / np.sqrt(v + EPS)
        geo = LAYERS[l]
        co, W = geo["co"], geo["W"]
        if l <= 4:
            # threshold: bn(y_conv) > 0  <=>  psum + bias > 0 (after folding)
            c = (b - m) + be / s
            flip = np.where(s < 0, -1.0, 1.0)
            wb = wb * flip[:, None, None, None]
            c = c * flip
            if l == 1:
                wmat, kap = wb, np.zeros(co)
            elif l == 4:
                # L3's output comes from a DVE is_gt -> {0,1} domain
                wmat, kap = wb, np.zeros(co)
            else:
                wmat = wb * 0.5
                kap = 0.5 * wb.sum(axis=(1, 2, 3))
            parity = l in (2, 4)
            bias_ch = (kap + c).astype(np.float32)  # per channel, > 0 test
            M = _mwidth(W, co, parity)
            bias_vec = np.zeros((M, 1), np.float32)
            for ox in range(W):
                for c_o in range(co):
                    bias_vec[_mcol(ox, c_o, W, co, parity), 0] = bias_ch[c_o]
            if l == 3:
                bias_vec = -bias_vec  # device uses is_gt(psum, -bias)
            d[f"thr{l}"] = bias_vec
            d[f"wk{l}"] = _toeplitz(wmat.astype(np.float32), W, parity).astype(BF16)
        else:
            wmat = wb * 0.5
            kap = 0.5 * wb.sum(axis=(1, 2, 3))
            d["wk5"] = _toeplitz(wmat.astype(np.float32), W, False).astype(BF16)
            # head mean+bn5: z = (s5/49)*psum_sum + s5*(kap+b5-m5)+be5
            d["s5"] = (s / 49.0).astype(np.float32).reshape(-1, 1)
            d["b5"] = (s * (kap + b - m) + be).astype(np.float32).reshape(-1, 1)
    # channel-summing matrix for the 7x7 mean: [112=(ox,co), 16]
    S = np.zeros((7 * 16, 16), np.float32)
    for ox in range(7):
        for c_ in range(16):
            S[ox * 16 + c_, c_] = 1.0
    d["Ssum"] = S.astype(BF16)
    wl = np.sign(np.asarray(inputs["wl"], np.float64))  # [10, 16]
    bl = np.asarray(inputs["bl"], np.float64)
    d["whead"] = (wl.T * 0.5).astype(BF16)  # [16, 10] lhsT
    d["bhead"] = (bl + 0.5 * wl.sum(axis=1)).astype(np.float32).reshape(-1, 1)
    d["id10"] = np.eye(10, dtype=np.float32)
    return d


_CACHE = {}


def _build():
    from concourse import bacc, tile, mybir

    f32 = mybir.dt.float32
    bf16 = mybir.dt.bfloat16
    ACT = mybir.ActivationFunctionType
    ALU = mybir.AluOpType
    AX = mybir.AxisListType

    nc = bacc.Bacc("TRN2", num_devices=NCORE)

    xT = nc.dram_tensor("xT", (784, NPER), bf16, kind="ExternalInput")
    dr = {}
    for l in range(1, 6):
        geo = LAYERS[l]
        K = (geo["W"] + 2) * geo["ci"]
        M = _mwidth(geo["W"], geo["co"], l in (2, 4))
        dr[f"wk{l}"] = nc.dram_tensor(f"wk{l}", (3, K, M), bf16, kind="ExternalInput")
        if l <= 4:
            dr[f"thr{l}"] = nc.dram_tensor(f"thr{l}", (M, 1), f32, kind="ExternalInput")
    dr["Ssum"] = nc.dram_tensor("Ssum", (112, 16), bf16, kind="ExternalInput")
    dr["whead"] = nc.dram_tensor("whead", (16, 10), bf16, kind="ExternalInput")
    dr["s5"] = nc.dram_tensor("s5", (16, 1), f32, kind="ExternalInput")
    dr["b5"] = nc.dram_tensor("b5", (16, 1), f32, kind="ExternalInput")
    dr["bhead"] = nc.dram_tensor("bhead", (10, 1), f32, kind="ExternalInput")
    dr["id10"] = nc.dram_tensor("id10", (10, 10), f32, kind="ExternalInput")
    out = nc.dram_tensor("out", (NPER, 10), f32, kind="ExternalOutput")

    with tile.TileContext(nc) as tc:
        stat = tc.alloc_tile_pool(name="stat", bufs=1)
        scr = tc.alloc_tile_pool(name="scr", bufs=4)
        ps = tc.alloc_tile_pool(name="ps", bufs=4, space="PSUM")
        ps_pm = tc.alloc_tile_pool(name="ps_pm", bufs=1, space="PSUM")
        ps_ph = tc.alloc_tile_pool(name="ps_ph", bufs=1, space="PSUM")
        ps_tr = tc.alloc_tile_pool(name="ps_tr", bufs=2, space="PSUM")

        # --- static buffers ---
        xb = stat.tile([128, 30 * T], bf16, tag="xb")
        a2 = stat.tile([128, 30 * T], bf16, tag="a2")
        a3 = stat.tile([128, 16 * T], bf16, tag="a3")
        a4 = stat.tile([128, 16 * T], bf16, tag="a4")
        a5b = stat.tile([128, 9 * T], bf16, tag="a5b")
        y5 = stat.tile([128, 7 * T], bf16, tag="y5")

        wt = {}
        for l in range(1, 6):
            geo = LAYERS[l]
            K = (geo["W"] + 2) * geo["ci"]
            M = _mwidth(geo["W"], geo["co"], l in (2, 4))
            for ky in range(3):
                t = stat.tile([128, 128], bf16, tag=f"w{l}_{ky}")
                if l == 1:
                    for b_ in range(4):
                        nc.sync.dma_start(t[32 * b_:32 * b_ + K, 0:M],
                                          dr[f"wk{l}"].ap()[ky, :, :])
                elif l == 3:
                    for b_ in range(2):
                        nc.sync.dma_start(t[64 * b_:64 * b_ + K, 0:M],
                                          dr[f"wk{l}"].ap()[ky, :, :])
                else:
                    nc.sync.dma_start(t[0:K, 0:M], dr[f"wk{l}"].ap()[ky, :, :])
                wt[(l, ky)] = t
        Ssb = stat.tile([128, 16], bf16, tag="Ssb")
        nc.sync.dma_start(Ssb[0:112, 0:16], dr["Ssum"].ap())
        whd = stat.tile([16, 16], bf16, tag="whd")
        nc.sync.dma_start(whd[0:16, 0:10], dr["whead"].ap())
        id10 = stat.tile([10, 16], f32, tag="id10")
        nc.sync.dma_start(id10[0:10, 0:10], dr["id10"].ap())
        cvec = {}
        for name, P in [("thr1", 112), ("thr2", 120), ("thr3", 112),
                        ("thr4", 120), ("s5", 16), ("b5", 16), ("bhead", 10)]:
            t = stat.tile([128, 1], f32, tag="c_" + name)
            nc.sync.dma_start(t[0:P, 0:1], dr[name].ap())
            cvec[name] = t

        # --- init: zero x pads, set tilde-domain buffers (pads) to -1 ---
        nc.scalar.memzero(xb[:, :])
        nc.scalar.memzero(a4[:, 0:16 * T])
        for buf, fp in [(a2, 30 * T), (a3, 16 * T), (a5b, 9 * T)]:
            nc.scalar.memzero(buf[:, 0:fp])
            nc.vector.tensor_scalar_add(buf[:, 0:fp], buf[:, 0:fp], -1.0)

        def conv_rows(l, src, nrows, psum_for_row):
            """Emit 3 accumulated matmuls per output row; returns list of psum tiles."""
            geo = LAYERS[l]
            K = (geo["W"] + 2) * geo["ci"]
            M = geo["W"] * geo["co"]
            pts = []
            for y in range(nrows):
                pt = psum_for_row()
                for ky in range(3):
                    nc.tensor.matmul(
                        out=pt[0:M, :],
                        lhsT=wt[(l, ky)][0:K, 0:M],
                        rhs=src[0:K, (y + ky) * T:(y + ky + 1) * T],
                        start=(ky == 0),
                        stop=(ky == 2),
                        tile_position=(0, 0),
                    )
                pts.append(pt)
            return pts

        for blk in range(NBLK):
            ioff = blk * T
            # ---- load + sign x ----
            src = xT.ap()[:, ioff:ioff + T].rearrange("(r c) n -> c r n", r=28)
            for b_ in range(4):
                dst = xb[32 * b_:32 * b_ + 28, T:29 * T].rearrange(
                    "p (r t) -> p r t", r=28)
                nc.sync.dma_start(dst, src)

            # ---- L1 ----
            for y in range(28):
                b_ = 32 * (y % 4)
                pt = ps.tile([128, T], f32, tag="pt")
                for ky in range(3):
                    nc.tensor.matmul(
                        out=pt[0:112, :], lhsT=wt[(1, ky)][b_:b_ + 30, 0:112],
                        rhs=xb[b_:b_ + 30, (y + ky) * T:(y + ky + 1) * T],
                        start=(ky == 0), stop=(ky == 2), tile_position=(b_, 0))
                nc.scalar.activation(
                    a2[0:112, (y + 1) * T:(y + 2) * T], pt[0:112, :],
                    ACT.Sign, bias=cvec["thr1"][0:112, 0:1])

            # ---- L2 (pool) ----
            for r in range(14):
                pt0 = ps.tile([128, T], f32, tag="pt")
                pt1 = ps.tile([128, T], f32, tag="pt")
                for (y, pt) in ((2 * r, pt0), (2 * r + 1, pt1)):
                    for ky in range(3):
                        nc.tensor.matmul(
                            out=pt[0:120, :], lhsT=wt[(2, ky)][0:120, 0:120],
                            rhs=a2[0:120, (y + ky) * T:(y + ky + 1) * T],
                            start=(ky == 0), stop=(ky == 2), tile_position=(0, 0))
                q0 = scr.tile([128, T], bf16, tag="q0")
                q1 = scr.tile([128, T], bf16, tag="q1")
                nc.scalar.activation(q0[0:120, :], pt0[0:120, :], ACT.Sign,
                                     bias=cvec["thr2"][0:120, 0:1])
                nc.scalar.activation(q1[0:120, :], pt1[0:120, :], ACT.Sign,
                                     bias=cvec["thr2"][0:120, 0:1])
                sy = scr.tile([128, T], bf16, tag="sy")
                nc.vector.tensor_tensor(out=sy[0:120, :], in0=q0[0:120, :],
                                        in1=q1[0:120, :], op=ALU.max)
                sq = scr.tile([128, T], bf16, tag="sq")
                nc.vector.tensor_copy(sq[0:56, :], sy[64:120, :])
                nc.vector.tensor_tensor(out=a3[0:56, (r + 1) * T:(r + 2) * T],
                                        in0=sy[0:56, :], in1=sq[0:56, :],
                                        op=ALU.max)

            # ---- L3 (2-way row tiling; replicate a3 to the upper half) ----
            nc.vector.tensor_copy(a3[64:128, :], a3[0:64, :])
            for y in range(14):
                b_ = 64 * (y % 2)
                pt = ps.tile([128, T], f32, tag="pt")
                for ky in range(3):
                    nc.tensor.matmul(
                        out=pt[0:112, :], lhsT=wt[(3, ky)][b_:b_ + 64, 0:112],
                        rhs=a3[b_:b_ + 64, (y + ky) * T:(y + ky + 1) * T],
                        start=(ky == 0), stop=(ky == 2), tile_position=(b_, 0))
                nc.vector.tensor_scalar(
                    out=a4[0:112, (y + 1) * T:(y + 2) * T], in0=pt[0:112, :],
                    scalar1=cvec["thr3"][0:112, 0:1], scalar2=None,
                    op0=ALU.is_gt)

            # ---- L4 (pool) ----
            for r in range(7):
                pt0 = ps.tile([128, T], f32, tag="pt")
                pt1 = ps.tile([128, T], f32, tag="pt")
                for (y, pt) in ((2 * r, pt0), (2 * r + 1, pt1)):
                    for ky in range(3):
                        nc.tensor.matmul(
                            out=pt[0:120, :], lhsT=wt[(4, ky)][0:128, 0:120],
                            rhs=a4[0:128, (y + ky) * T:(y + ky + 1) * T],
                            start=(ky == 0), stop=(ky == 2), tile_position=(0, 0))
                q0 = scr.tile([128, T], bf16, tag="q0")
                q1 = scr.tile([128, T], bf16, tag="q1")
                nc.scalar.activation(q0[0:120, :], pt0[0:120, :], ACT.Sign,
                                     bias=cvec["thr4"][0:120, 0:1])
                nc.scalar.activation(q1[0:120, :], pt1[0:120, :], ACT.Sign,
                                     bias=cvec["thr4"][0:120, 0:1])
                sy = scr.tile([128, T], bf16, tag="sy")
                nc.vector.tensor_tensor(out=sy[0:120, :], in0=q0[0:120, :],
                                        in1=q1[0:120, :], op=ALU.max)
                sq = scr.tile([128, T], bf16, tag="sq")
                nc.vector.tensor_copy(sq[0:56, :], sy[64:120, :])
                nc.vector.tensor_tensor(out=a5b[0:56, (r + 1) * T:(r + 2) * T],
                                        in0=sy[0:56, :], in1=sq[0:56, :],
                                        op=ALU.max)

            # ---- L5 conv -> y5 (raw values, bf16-exact halves) ----
            for y in range(7):
                pt = ps.tile([128, T], f32, tag="pt")
                for ky in range(3):
                    nc.tensor.matmul(
                        out=pt[0:112, :], lhsT=wt[(5, ky)][0:72, 0:112],
                        rhs=a5b[0:72, (y + ky) * T:(y + ky + 1) * T],
                        start=(ky == 0), stop=(ky == 2), tile_position=(0, 0))
                nc.vector.tensor_copy(y5[0:112, y * T:(y + 1) * T],
                                      pt[0:112, :])

            # ---- head: mean over 7x7 via 7 accumulated matmuls ----
            pm = ps_pm.tile([128, T], f32, tag="pm")
            for r in range(7):
                nc.tensor.matmul(
                    out=pm[0:16, :], lhsT=Ssb[0:112, 0:16],
                    rhs=y5[0:112, r * T:(r + 1) * T],
                    start=(r == 0), stop=(r == 6), tile_position=(0, 0))
            u = scr.tile([16, T], bf16, tag="u")
            nc.scalar.activation(u[0:16, :], pm[0:16, :], ACT.Sign,
                                 bias=cvec["b5"][0:16, 0:1],
                                 scale=cvec["s5"][0:16, 0:1])
            ph = ps_ph.tile([128, T], f32, tag="ph")
            nc.tensor.matmul(out=ph[0:10, :], lhsT=whd[0:16, 0:10],
                             rhs=u[0:16, :], start=True, stop=True,
                             tile_position=(0, 0))
            hh = scr.tile([16, T], f32, tag="hh")
            nc.scalar.activation(hh[0:10, :], ph[0:10, :], ACT.Identity,
                                 bias=cvec["bhead"][0:10, 0:1])

            for k in range(T // 128):
                ptr = ps_tr.tile([128, 16], f32, tag="ptr")
                nc.tensor.transpose(ptr[0:128, 0:10],
                                    hh[0:10, k * 128:(k + 1) * 128],
                                    id10[0:10, 0:10])
                mx = scr.tile([128, 1], f32, tag="mx")
                nc.vector.tensor_reduce(mx[0:128, 0:1], ptr[0:128, 0:10],
                                        axis=AX.X, op=ALU.max, negate=True)
                ex = scr.tile([128, 16], f32, tag="ex")
                nc.scalar.activation(ex[0:128, 0:10], ptr[0:128, 0:10],
                                     ACT.Exp, bias=mx[0:128, 0:1])
                sm = scr.tile([128, 1], f32, tag="sm")
                nc.vector.tensor_reduce(sm[0:128, 0:1], ex[0:128, 0:10],
                                        axis=AX.X, op=ALU.add)
                lg = scr.tile([128, 1], f32, tag="lg")
                nc.scalar.activation(lg[0:128, 0:1], sm[0:128, 0:1], ACT.Ln)
                tt = scr.tile([128, 1], f32, tag="tt")
                nc.vector.tensor_tensor(out=tt[0:128, 0:1], in0=mx[0:128, 0:1],
                                        in1=lg[0:128, 0:1], op=ALU.subtract)
                osb = scr.tile([128, 16], f32, tag="osb")
                nc.scalar.activation(osb[0:128, 0:10], ptr[0:128, 0:10],
                                     ACT.Identity, bias=tt[0:128, 0:1])
                row0 = ioff + k * 128
                nc.sync.dma_start(out.ap()[row0:row0 + 128, 0:10],
                                  osb[0:128, 0:10])

        for p in (ps_tr, ps_ph, ps_pm, ps, scr, stat):
            p.release()

    nc.compile()
    return nc


def kernel(**inputs):
    from concourse.bass_utils import run_bass_kernel_spmd
    import os

    if "nc" not in _CACHE:
        _CACHE["nc"] = _build()
    nc = _CACHE["nc"]

    folded = _host_fold(inputs)
    x = np.asarray(inputs["x"], np.float32).reshape(8192, 784)
    xT_full = np.ascontiguousarray(np.sign(x).T.astype(BF16))  # [784, 8192]

    in_maps = []
    for i in range(NCORE):
        m = {"xT": np.ascontiguousarray(xT_full[:, i * NPER:(i + 1) * NPER])}
        for k, v in folded.items():
            m[k] = v
        in_maps.append(m)

    res = run_bass_kernel_spmd(nc, in_maps, core_ids=list(range(NCORE)))
    _CACHE["last_result"] = res
    outs = [res.results[i]["out"] for i in range(NCORE)]
    return np.concatenate(outs, axis=0).astype(np.float32)


# revision 18
# speedup vs baseline: 1.2655x; 1.2581x over previous
"""Bass/Trainium2 kernel for binarized AlexNet-OWT-BN (MNIST-shaped), 8-core data parallel.

Contract: kernel(**inputs) takes the FULL unsharded inputs (x: [8192,1,28,28] f32
plus conv/bn/linear params) and returns the FULL [8192,10] f32 log-softmax output.

Design notes
------------
- Data parallel: batch 8192 -> 8 cores x 1024 images; each core runs 2 blocks of
  T=512 images (512 = fp32 PSUM bank limit for the matmul moving dim).
- The host ships sign(x) as bf16 (input staging, like the binarized weights) and
  folds conv bias + BN + ReLU + binarize of every layer into per-channel
  threshold tests applied directly to the conv PSUM: layers 1/2/4 via one
  ScalarE Sign (+-1 "tilde" activation domain, halved next-layer weights,
  0.5*sum(w) folded into the threshold, pad slots = -1), layer 3 via a DVE
  tensor_scalar is_gt ({0,1} domain, unhalved L4 weights, pad slots = 0) --
  splitting threshold work across both engines to balance them.
- BN channels with negative scale are handled by negating that channel's
  weights and threshold on the host, so max-pooling is always correct.
- Conv as matmul: partitions hold one padded image row (padded_ix x ci <= 128),
  M = out_width x co <= 128 (block-Toeplitz weights built on host), free dim =
  images; the 3 ky taps are PSUM-accumulated matmuls whose rhs differs only by
  a free-dim offset (rows live in the free dim: (padded_row, image)).
- 2x2 maxpool runs on the binarized SBUF values (PSUM allows only one
  tensor_tensor operand, and bf16 ops get the 2x DVE mode): y-pairs max
  lane-aligned, x-pairs use a parity-major M layout (even px at partitions
  0-55, odd px at 64-119) so one quadrant-aligned copy + one aligned max.
- conv1/conv3 use 4x/2x TensorE row tiling (tile_position row bands; conv1
  input bands come straight from a 4-way DMA fan-out, conv3's from one DVE
  copy) since K=30/64 << 128.
- Head: 7 accumulated matmuls against a channel-summing matrix implement the
  7x7 mean; Sign gives the binarized features; a 16->10 matmul, TensorE
  transpose per 128-image chunk, and a free-dim log-softmax finish.
"""

import sys

sys.path.insert(0, "/opt/trn_rl_repo")

import numpy as np
import ml_dtypes

EPS = 1e-5
T = 512
NBLK = 2
NCORE = 8
NPER = T * NBLK  # images per core

BF16 = ml_dtypes.bfloat16

# layer geometry: (ci, co, W_out, pooled)
LAYERS = {
    1: dict(ci=1, co=4, W=28),
    2: dict(ci=4, co=4, W=28),
    3: dict(ci=4, co=8, W=14),
    4: dict(ci=8, co=8, W=14),
    5: dict(ci=8, co=16, W=7),
}


def _krow(ix, c, W, ci):
    """K-partition index for padded column ix: real px first, pads at the end."""
    if ix == 0:
        return W * ci + c
    if ix == W + 1:
        return W * ci + ci + c
    return (ix - 1) * ci + c


def _mcol(ox, c, W, co, parity):
    """M column for out px ox; parity-major (evens/gap/odds) for pool layers."""
    if not parity:
        return ox * co + c
    half = (W // 2) * co
    pad_half = ((half + 31) // 32) * 32  # odds start at the next quadrant
    if ox % 2 == 0:
        return (ox // 2) * co + c
    return pad_half + (ox // 2) * co + c


def _mwidth(W, co, parity):
    if not parity:
        return W * co
    half = (W // 2) * co
    pad_half = ((half + 31) // 32) * 32
    return pad_half + half


def _toeplitz(wmat, W, parity):
    co, ci = wmat.shape[0], wmat.shape[1]
    K = (W + 2) * ci
    M = _mwidth(W, co, parity)
    out = []
    for ky in range(3):
        Wk = np.zeros((K, M), np.float32)
        for ox in range(W):
            for kx in range(3):
                ix = ox + kx
                for c_o in range(co):
                    for c_i in range(ci):
                        Wk[_krow(ix, c_i, W, ci),
                           _mcol(ox, c_o, W, co, parity)] = wmat[c_o, c_i, ky, kx]
        out.append(Wk)
    return np.stack(out)  # [3, K, M]


def _host_fold(inputs):
    """Fold weights + BN into Toeplitz matmul weights and threshold vectors."""
    d = {}
    for l in range(1, 6):
        tag = str(l)
        w = np.asarray(inputs["w" + tag], np.float64)
        b = np.asarray(inputs["b" + tag], np.float64)
        g = np.asarray(inputs["g" + tag], np.float64)
        be = np.asarray(inputs["be" + tag], np.float64)
        m = np.asarray(inputs["m" + tag], np.float64)
        v = np.asarray(inputs["v" + tag], np.float64)
        wb = np.sign(w).astype(np.float64)
        s = g / np.sqrt(v + EPS)
        geo = LAYERS[l]
        co, W = geo["co"], geo["W"]
        if l <= 4:
            # threshold: bn(y_conv) > 0  <=>  psum + bias > 0 (after folding)
            c = (b - m) + be / s
            flip = np.where(s < 0, -1.0, 1.0)
            wb = wb * flip[:, None, None, None]
            c = c * flip
            if l == 1:
                wmat, kap = wb, np.zeros(co)
            elif l == 4:
                # L3's output comes from a DVE is_gt -> {0,1} domain
                wmat, kap = wb, np.zeros(co)
            else:
                wmat = wb * 0.5
                kap = 0.5 * wb.sum(axis=(1, 2, 3))
            parity = l in (2, 4)
            bias_ch = (kap + c).astype(np.float32)  # per channel, > 0 test
            M = _mwidth(W, co, parity)
            bias_vec = np.zeros((M, 1), np.float32)
            for ox in range(W):
                for c_o in range(co):
                    bias_vec[_mcol(ox, c_o, W, co, parity), 0] = bias_ch[c_o]
            if l == 3:
                bias_vec = -bias_vec  # device uses is_gt(psum, -bias)
            d[f"thr{l}"] = bias_vec
            d[f"wk{l}"] = _toeplitz(wmat.astype(np.float32), W, parity).astype(BF16)
        else:
            wmat = wb * 0.5
            kap = 0.5 * wb.sum(axis=(1, 2, 3))
            d["wk5"] = _toeplitz(wmat.astype(np.float32), W, False).astype(BF16)
            # head mean+bn5: z = (s5/49)*psum_sum + s5*(kap+b5-m5)+be5
            d["s5"] = (s / 49.0).astype(np.float32).reshape(-1, 1)
            d["b5"] = (s * (kap + b - m) + be).astype(np.float32).reshape(-1, 1)
    # channel-summing matrix for the 7x7 mean: [112=(ox,co), 16]
    S = np.zeros((7 * 16, 16), np.float32)
    for ox in range(7):
        for c_ in range(16):
            S[ox * 16 + c_, c_] = 1.0
    d["Ssum"] = S.astype(BF16)
    wl = np.sign(np.asarray(inputs["wl"], np.float64))  # [10, 16]
    bl = np.asarray(inputs["bl"], np.float64)
    d["whead"] = (wl.T * 0.5).astype(BF16)  # [16, 10] lhsT
    d["bhead"] = (bl + 0.5 * wl.sum(axis=1)).astype(np.float32).reshape(-1, 1)
    d["id10"] = np.eye(10, dtype=np.float32)
    return d


_CACHE = {}


def _build():
    from concourse import bacc, tile, mybir

    f32 = mybir.dt.float32
    bf16 = mybir.dt.bfloat16
    ACT = mybir.ActivationFunctionType
    ALU = mybir.AluOpType
    AX = mybir.AxisListType

    nc = bacc.Bacc("TRN2", num_devices=NCORE)

    xT = nc.dram_tensor("xT", (784, NPER), bf16, kind="ExternalInput")
    dr = {}
    for l in range(1, 6):
        geo = LAYERS[l]
        K = (geo["W"] + 2) * geo["ci"]
        M = _mwidth(geo["W"], geo["co"], l in (2, 4))
        dr[f"wk{l}"] = nc.dram_tensor(f"wk{l}", (3, K, M), bf16, kind="ExternalInput")
        if l <= 4:
            dr[f"thr{l}"] = nc.dram_tensor(f"thr{l}", (M, 1), f32, kind="ExternalInput")
    dr["Ssum"] = nc.dram_tensor("Ssum", (112, 16), bf16, kind="ExternalInput")
    dr["whead"] = nc.dram_tensor("whead", (16, 10), bf16, kind="ExternalInput")
    dr["s5"] = nc.dram_tensor("s5", (16, 1), f32, kind="ExternalInput")
    dr["b5"] = nc.dram_tensor("b5", (16, 1), f32, kind="ExternalInput")
    dr["bhead"] = nc.dram_tensor("bhead", (10, 1), f32, kind="ExternalInput")
    dr["id10"] = nc.dram_tensor("id10", (10, 10), f32, kind="ExternalInput")
    out = nc.dram_tensor("out", (NPER, 10), f32, kind="ExternalOutput")

    with tile.TileContext(nc) as tc:
        stat = tc.alloc_tile_pool(name="stat", bufs=1)
        scr = tc.alloc_tile_pool(name="scr", bufs=4)
        ps = tc.alloc_tile_pool(name="ps", bufs=4, space="PSUM")
        ps_pm = tc.alloc_tile_pool(name="ps_pm", bufs=1, space="PSUM")
        ps_ph = tc.alloc_tile_pool(name="ps_ph", bufs=1, space="PSUM")
        ps_tr = tc.alloc_tile_pool(name="ps_tr", bufs=2, space="PSUM")

        # --- static buffers ---
        xb = stat.tile([128, 30 * T], bf16, tag="xb")
        a2 = stat.tile([128, 30 * T], bf16, tag="a2")
        a3 = stat.tile([128, 16 * T], bf16, tag="a3")
        a4 = stat.tile([128, 16 * T], bf16, tag="a4")
        a5b = stat.tile([128, 9 * T], bf16, tag="a5b")
        y5 = stat.tile([128, 7 * T], bf16, tag="y5")

        wt = {}
        for l in range(1, 6):
            geo = LAYERS[l]
            K = (geo["W"] + 2) * geo["ci"]
            M = _mwidth(geo["W"], geo["co"], l in (2, 4))
            for ky in range(3):
                t = stat.tile([128, 128], bf16, tag=f"w{l}_{ky}")
                if l == 1:
                    for b_ in range(4):
                        nc.sync.dma_start(t[32 * b_:32 * b_ + K, 0:M],
                                          dr[f"wk{l}"].ap()[ky, :, :])
                elif l == 3:
                    for b_ in range(2):
                        nc.sync.dma_start(t[64 * b_:64 * b_ + K, 0:M],
                                          dr[f"wk{l}"].ap()[ky, :, :])
                else:
                    nc.sync.dma_start(t[0:K, 0:M], dr[f"wk{l}"].ap()[ky, :, :])
                wt[(l, ky)] = t
        Ssb = stat.tile([128, 16], bf16, tag="Ssb")
        nc.sync.dma_start(Ssb[0:112, 0:16], dr["Ssum"].ap())
        whd = stat.tile([16, 16], bf16, tag="whd")
        nc.sync.dma_start(whd[0:16, 0:10], dr["whead"].ap())
        id10 = stat.tile([10, 16], f32, tag="id10")
        nc.sync.dma_start(id10[0:10, 0:10], dr["id10"].ap())
        cvec = {}
        for name, P in [("thr1", 112), ("thr2", 120), ("thr3", 112),
                        ("thr4", 120), ("s5", 16), ("b5", 16), ("bhead", 10)]:
            t = stat.tile([128, 1], f32, tag="c_" + name)
            nc.sync.dma_start(t[0:P, 0:1], dr[name].ap())
            cvec[name] = t

        # --- init: zero x pads, set tilde-domain buffers (pads) to -1 ---
        nc.scalar.memzero(xb[:, :])
        nc.scalar.memzero(a4[:, 0:16 * T])
        for buf, fp in [(a2, 30 * T), (a3, 16 * T), (a5b, 9 * T)]:
            nc.scalar.memzero(buf[:, 0:fp])
            nc.vector.tensor_scalar_add(buf[:, 0:fp], buf[:, 0:fp], -1.0)

        def conv_rows(l, src, nrows, psum_for_row):
            """Emit 3 accumulated matmuls per output row; returns list of psum tiles."""
            geo = LAYERS[l]
            K = (geo["W"] + 2) * geo["ci"]
            M = geo["W"] * geo["co"]
            pts = []
            for y in range(nrows):
                pt = psum_for_row()
                for ky in range(3):
                    nc.tensor.matmul(
                        out=pt[0:M, :],
                        lhsT=wt[(l, ky)][0:K, 0:M],
                        rhs=src[0:K, (y + ky) * T:(y + ky + 1) * T],
                        start=(ky == 0),
                        stop=(ky == 2),
                        tile_position=(0, 0),
                    )
                pts.append(pt)
            return pts

        for blk in range(NBLK):
            ioff = blk * T
            # ---- load + sign x ----
            src = xT.ap()[:, ioff:ioff + T].rearrange("(r c) n -> c r n", r=28)
            for b_ in range(4):
                dst = xb[32 * b_:32 * b_ + 28, T:29 * T].rearrange(
                    "p (r t) -> p r t", r=28)
                nc.sync.dma_start(dst, src)

            # ---- L1 ----
            for y in range(28):
                b_ = 32 * (y % 4)
                pt = ps.tile([128, T], f32, tag="pt")
                for ky in range(3):
                    nc.tensor.matmul(
                        out=pt[0:112, :], lhsT=wt[(1, ky)][b_:b_ + 30, 0:112],
                        rhs=xb[b_:b_ + 30, (y + ky) * T:(y + ky + 1) * T],
                        start=(ky == 0), stop=(ky == 2), tile_position=(b_, 0))
                nc.scalar.activation(
                    a2[0:112, (y + 1) * T:(y + 2) * T], pt[0:112, :],
                    ACT.Sign, bias=cvec["thr1"][0:112, 0:1])

            # ---- L2 (pool) ----
            for r in range(14):
                pt0 = ps.tile([128, T], f32, tag="pt")
                pt1 = ps.tile([128, T], f32, tag="pt")
                for (y, pt) in ((2 * r, pt0), (2 * r + 1, pt1)):
                    for ky in range(3):
                        nc.tensor.matmul(
                            out=pt[0:120, :], lhsT=wt[(2, ky)][0:120, 0:120],
                            rhs=a2[0:120, (y + ky) * T:(y + ky + 1) * T],
                            start=(ky == 0), stop=(ky == 2), tile_position=(0, 0))
                q0 = scr.tile([128, T], bf16, tag="q0")
                q1 = scr.tile([128, T], bf16, tag="q1")
                nc.scalar.activation(q0[0:120, :], pt0[0:120, :], ACT.Sign,
                                     bias=cvec["thr2"][0:120, 0:1])
                nc.scalar.activation(q1[0:120, :], pt1[0:120, :], ACT.Sign,
                                     bias=cvec["thr2"][0:120, 0:1])
                sy = scr.tile([128, T], bf16, tag="sy")
                nc.vector.tensor_tensor(out=sy[0:120, :], in0=q0[0:120, :],
                                        in1=q1[0:120, :], op=ALU.max)
                sq = scr.tile([128, T], bf16, tag="sq")
                nc.vector.tensor_copy(sq[0:56, :], sy[64:120, :])
                nc.vector.tensor_tensor(out=a3[0:56, (r + 1) * T:(r + 2) * T],
                                        in0=sy[0:56, :], in1=sq[0:56, :],
                                        op=ALU.max)

            # ---- L3 (2-way row tiling; replicate a3 to the upper half) ----
            nc.vector.tensor_copy(a3[64:128, :], a3[0:64, :])
            for y in range(14):
                b_ = 64 * (y % 2)
                pt = ps.tile([128, T], f32, tag="pt")
                for ky in range(3):
                    nc.tensor.matmul(
                        out=pt[0:112, :], lhsT=wt[(3, ky)][b_:b_ + 64, 0:112],
                        rhs=a3[b_:b_ + 64, (y + ky) * T:(y + ky + 1) * T],
                        start=(ky == 0), stop=(ky == 2), tile_position=(b_, 0))
                nc.vector.tensor_scalar(
                    out=a4[0:112, (y + 1) * T:(y + 2) * T], in0=pt[0:112, :],
                    scalar1=cvec["thr3"][0:112, 0:1], scalar2=None,
                    op0=ALU.is_gt)

            # ---- L4 (pool) ----
            for r in range(7):
                pt0 = ps.tile([128, T], f32, tag="pt")
                pt1 = ps.tile([128, T], f32, tag="pt")
                for (y, pt) in ((2 * r, pt0), (2 * r + 1, pt1)):
                    for ky in range(3):
                        nc.tensor.matmul(
                            out=pt[0:120, :], lhsT=wt[(4, ky)][0:128, 0:120],
                            rhs=a4[0:128, (y + ky) * T:(y + ky + 1) * T],
                            start=(ky == 0), stop=(ky == 2), tile_position=(0, 0))
                q0 = scr.tile([128, T], bf16, tag="q0")
                q1 = scr.tile([128, T], bf16, tag="q1")
                nc.scalar.activation(q0[0:120, :], pt0[0:120, :], ACT.Sign,
                                     bias=cvec["thr4"][0:120, 0:1])
                nc.scalar.activation(q1[0:120, :], pt1[0:120, :], ACT.Sign,
                                     bias=cvec["thr4"][0:120, 0:1])
                sy = scr.tile([128, T], bf16, tag="sy")
                nc.vector.tensor_tensor(out=sy[0:120, :], in0=q0[0:120, :],
                                        in1=q1[0:120, :], op=ALU.max)
                sq = scr.tile([128, T], bf16, tag="sq")
                nc.vector.tensor_copy(sq[0:56, :], sy[64:120, :])
                nc.vector.tensor_tensor(out=a5b[0:56, (r + 1) * T:(r + 2) * T],
                                        in0=sy[0:56, :], in1=sq[0:56, :],
                                        op=ALU.max)

            # ---- L5 conv -> y5 (raw values, bf16-exact halves) ----
            for y in range(7):
                pt = ps.tile([128, T], f32, tag="pt")
                for ky in range(3):
                    nc.tensor.matmul(
                        out=pt[0:112, :], lhsT=wt[(5, ky)][0:72, 0:112],
                        rhs=a5b[0:72, (y + ky) * T:(y + ky + 1) * T],
                        start=(ky == 0), stop=(ky == 2), tile_position=(0, 0))
                nc.vector.tensor_copy(y5[0:112, y * T:(y + 1) * T],
                                      pt[0:112, :])

            # ---- head: mean over 7x7 via 7 accumulated matmuls ----
            pm = ps_pm.tile([128, T], f32, tag="pm")
            for r in range(7):
                nc.tensor.matmul(
                    out=pm[0:16, :], lhsT=Ssb[0:112, 0:16],
                    rhs=y5[0:112, r * T:(r + 1) * T],
                    start=(r == 0), stop=(r == 6), tile_position=(0, 0))
            u = scr.tile([16, T], bf16, tag="u")
            nc.scalar.activation(u[0:16, :], pm[0:16, :], ACT.Sign,
                                 bias=cvec["b5"][0:16, 0:1],
                                 scale=cvec["s5"][0:16, 0:1])
            ph = ps_ph.tile([128, T], f32, tag="ph")
            nc.tensor.matmul(out=ph[0:10, :], lhsT=whd[0:16, 0:10],
                             rhs=u[0:16, :], start=True, stop=True,
                             tile_position=(0, 0))
            hh = scr.tile([16, T], f32, tag="hh")
            nc.scalar.activation(hh[0:10, :], ph[0:10, :], ACT.Identity,
                                 bias=cvec["bhead"][0:10, 0:1])

            for k in range(T // 128):
                ptr = ps_tr.tile([128, 16], f32, tag="ptr")
                nc.tensor.transpose(ptr[0:128, 0:10],
                                    hh[0:10, k * 128:(k + 1) * 128],
                                    id10[0:10, 0:10])
                mx = scr.tile([128, 1], f32, tag="mx")
                nc.vector.tensor_reduce(mx[0:128, 0:1], ptr[0:128, 0:10],
                                        axis=AX.X, op=ALU.max, negate=True)
                ex = scr.tile([128, 16], f32, tag="ex")
                nc.scalar.activation(ex[0:128, 0:10], ptr[0:128, 0:10],
                                     ACT.Exp, bias=mx[0:128, 0:1])
                sm = scr.tile([128, 1], f32, tag="sm")
                nc.vector.tensor_reduce(sm[0:128, 0:1], ex[0:128, 0:10],
                                        axis=AX.X, op=ALU.add)
                lg = scr.tile([128, 1], f32, tag="lg")
                nc.scalar.activation(lg[0:128, 0:1], sm[0:128, 0:1], ACT.Ln)
                tt = scr.tile([128, 1], f32, tag="tt")
                nc.vector.tensor_tensor(out=tt[0:128, 0:1], in0=mx[0:128, 0:1],
                                        in1=lg[0:128, 0:1], op=ALU.subtract)
                osb = scr.tile([128, 16], f32, tag="osb")
                nc.scalar.activation(osb[0:128, 0:10], ptr[0:128, 0:10],
                                     ACT.Identity, bias=tt[0:128, 0:1])
                row0 = ioff + k * 128
                nc.sync.dma_start(out.ap()[row0:row0 + 128, 0:10],
                                  osb[0:128, 0:10])

        for p in (ps_tr, ps_ph, ps_pm, ps, scr, stat):
            p.release()

    nc.compile()
    return nc


def kernel(**inputs):
    from concourse.bass_utils import run_bass_kernel_spmd
    import os

    if "nc" not in _CACHE:
        _CACHE["nc"] = _build()
    nc = _CACHE["nc"]

    folded = _host_fold(inputs)
    x = np.asarray(inputs["x"], np.float32).reshape(8192, 784)
    xT_full = np.ascontiguousarray(np.sign(x).T.astype(BF16))  # [784, 8192]

    in_maps = []
    for i in range(NCORE):
        m = {"xT": np.ascontiguousarray(xT_full[:, i * NPER:(i + 1) * NPER])}
        for k, v in folded.items():
            m[k] = v
        in_maps.append(m)

    res = run_bass_kernel_spmd(nc, in_maps, core_ids=list(range(NCORE)))
    _CACHE["last_result"] = res
    outs = [res.results[i]["out"] for i in range(NCORE)]
    return np.concatenate(outs, axis=0).astype(np.float32)


# revision 19
# speedup vs baseline: 1.2718x; 1.0050x over previous
"""Bass/Trainium2 kernel for binarized AlexNet-OWT-BN (MNIST-shaped), 8-core data parallel.

Contract: kernel(**inputs) takes the FULL unsharded inputs (x: [8192,1,28,28] f32
plus conv/bn/linear params) and returns the FULL [8192,10] f32 log-softmax output.

Design notes
------------
- Data parallel: batch 8192 -> 8 cores x 1024 images; each core runs 2 blocks of
  T=512 images (512 = fp32 PSUM bank limit for the matmul moving dim).
- The host ships sign(x) as bf16 (input staging, like the binarized weights) and
  folds conv bias + BN + ReLU + binarize of every layer into per-channel
  threshold tests applied directly to the conv PSUM: layers 1/2/4 via one
  ScalarE Sign (+-1 "tilde" activation domain, halved next-layer weights,
  0.5*sum(w) folded into the threshold, pad slots = -1), layer 3 via a DVE
  tensor_scalar is_gt ({0,1} domain, unhalved L4 weights, pad slots = 0) --
  splitting threshold work across both engines to balance them.
- BN channels with negative scale are handled by negating that channel's
  weights and threshold on the host, so max-pooling is always correct.
- Conv as matmul: partitions hold one padded image row (padded_ix x ci <= 128),
  M = out_width x co <= 128 (block-Toeplitz weights built on host), free dim =
  images; the 3 ky taps are PSUM-accumulated matmuls whose rhs differs only by
  a free-dim offset (rows live in the free dim: (padded_row, image)).
- 2x2 maxpool runs on the binarized SBUF values (PSUM allows only one
  tensor_tensor operand, and bf16 ops get the 2x DVE mode): y-pairs max
  lane-aligned, x-pairs use a parity-major M layout (even px at partitions
  0-55, odd px at 64-119) so one quadrant-aligned copy + one aligned max.
- conv1/conv3 use 4x/2x TensorE row tiling (tile_position row bands; conv1
  input bands come straight from a 4-way DMA fan-out, conv3's from one DVE
  copy) since K=30/64 << 128.
- Head: 7 accumulated matmuls against a channel-summing matrix implement the
  7x7 mean; Sign gives the binarized features; a 16->10 matmul, TensorE
  transpose per 128-image chunk, and a free-dim log-softmax finish.
"""

import sys

sys.path.insert(0, "/opt/trn_rl_repo")

import numpy as np
import ml_dtypes

EPS = 1e-5
T = 512
NBLK = 2
NCORE = 8
NPER = T * NBLK  # images per core

BF16 = ml_dtypes.bfloat16

# layer geometry: (ci, co, W_out, pooled)
LAYERS = {
    1: dict(ci=1, co=4, W=28),
    2: dict(ci=4, co=4, W=28),
    3: dict(ci=4, co=8, W=14),
    4: dict(ci=8, co=8, W=14),
    5: dict(ci=8, co=16, W=7),
}


def _krow(ix, c, W, ci):
    """K-partition index for padded column ix: real px first, pads at the end."""
    if ix == 0:
        return W * ci + c
    if ix == W + 1:
        return W * ci + ci + c
    return (ix - 1) * ci + c


def _mcol(ox, c, W, co, parity):
    """M column for out px ox; parity-major (evens/gap/odds) for pool layers."""
    if not parity:
        return ox * co + c
    half = (W // 2) * co
    pad_half = ((half + 31) // 32) * 32  # odds start at the next quadrant
    if ox % 2 == 0:
        return (ox // 2) * co + c
    return pad_half + (ox // 2) * co + c


def _mwidth(W, co, parity):
    if not parity:
        return W * co
    half = (W // 2) * co
    pad_half = ((half + 31) // 32) * 32
    return pad_half + half


def _toeplitz(wmat, W, parity):
    co, ci = wmat.shape[0], wmat.shape[1]
    K = (W + 2) * ci
    M = _mwidth(W, co, parity)
    out = []
    for ky in range(3):
        Wk = np.zeros((K, M), np.float32)
        for ox in range(W):
            for kx in range(3):
                ix = ox + kx
                for c_o in range(co):
                    for c_i in range(ci):
                        Wk[_krow(ix, c_i, W, ci),
                           _mcol(ox, c_o, W, co, parity)] = wmat[c_o, c_i, ky, kx]
        out.append(Wk)
    return np.stack(out)  # [3, K, M]


def _host_fold(inputs):
    """Fold weights + BN into Toeplitz matmul weights and threshold vectors."""
    d = {}
    for l in range(1, 6):
        tag = str(l)
        w = np.asarray(inputs["w" + tag], np.float64)
        b = np.asarray(inputs["b" + tag], np.float64)
        g = np.asarray(inputs["g" + tag], np.float64)
        be = np.asarray(inputs["be" + tag], np.float64)
        m = np.asarray(inputs["m" + tag], np.float64)
        v = np.asarray(inputs["v" + tag], np.float64)
        wb = np.sign(w).astype(np.float64)
        s = g / np.sqrt(v + EPS)
        geo = LAYERS[l]
        co, W = geo["co"], geo["W"]
        if l <= 4:
            # threshold: bn(y_conv) > 0  <=>  psum + bias > 0 (after folding)
            c = (b - m) + be / s
            flip = np.where(s < 0, -1.0, 1.0)
            wb = wb * flip[:, None, None, None]
            c = c * flip
            if l == 1:
                wmat, kap = wb, np.zeros(co)
            elif l == 4:
                # L3's output comes from a DVE is_gt -> {0,1} domain
                wmat, kap = wb, np.zeros(co)
            else:
                wmat = wb * 0.5
                kap = 0.5 * wb.sum(axis=(1, 2, 3))
            parity = l in (2, 4)
            bias_ch = (kap + c).astype(np.float32)  # per channel, > 0 test
            M = _mwidth(W, co, parity)
            bias_vec = np.zeros((M, 1), np.float32)
            for ox in range(W):
                for c_o in range(co):
                    bias_vec[_mcol(ox, c_o, W, co, parity), 0] = bias_ch[c_o]
            if l == 3:
                bias_vec = -bias_vec  # device uses is_gt(psum, -bias)
            d[f"thr{l}"] = bias_vec
            d[f"wk{l}"] = _toeplitz(wmat.astype(np.float32), W, parity).astype(BF16)
        else:
            wmat = wb * 0.5
            kap = 0.5 * wb.sum(axis=(1, 2, 3))
            d["wk5"] = _toeplitz(wmat.astype(np.float32), W, False).astype(BF16)
            # head mean+bn5: z = (s5/49)*psum_sum + s5*(kap+b5-m5)+be5
            d["s5"] = (s / 49.0).astype(np.float32).reshape(-1, 1)
            d["b5"] = (s * (kap + b - m) + be).astype(np.float32).reshape(-1, 1)
    # channel-summing matrix for the 7x7 mean: [112=(ox,co), 16]
    S = np.zeros((7 * 16, 16), np.float32)
    for ox in range(7):
        for c_ in range(16):
            S[ox * 16 + c_, c_] = 1.0
    d["Ssum"] = S.astype(BF16)
    wl = np.sign(np.asarray(inputs["wl"], np.float64))  # [10, 16]
    bl = np.asarray(inputs["bl"], np.float64)
    d["whead"] = (wl.T * 0.5).astype(BF16)  # [16, 10] lhsT
    d["bhead"] = (bl + 0.5 * wl.sum(axis=1)).astype(np.float32).reshape(-1, 1)
    d["id10"] = np.eye(10, dtype=np.float32)
    return d


_CACHE = {}


def _build():
    from concourse import bacc, tile, mybir

    f32 = mybir.dt.float32
    bf16 = mybir.dt.bfloat16
    ACT = mybir.ActivationFunctionType
    ALU = mybir.AluOpType
    AX = mybir.AxisListType

    nc = bacc.Bacc("TRN2", num_devices=NCORE)

    xT = nc.dram_tensor("xT", (784, NPER), bf16, kind="ExternalInput")
    dr = {}
    for l in range(1, 6):
        geo = LAYERS[l]
        K = (geo["W"] + 2) * geo["ci"]
        M = _mwidth(geo["W"], geo["co"], l in (2, 4))
        dr[f"wk{l}"] = nc.dram_tensor(f"wk{l}", (3, K, M), bf16, kind="ExternalInput")
        if l <= 4:
            dr[f"thr{l}"] = nc.dram_tensor(f"thr{l}", (M, 1), f32, kind="ExternalInput")
    dr["Ssum"] = nc.dram_tensor("Ssum", (112, 16), bf16, kind="ExternalInput")
    dr["whead"] = nc.dram_tensor("whead", (16, 10), bf16, kind="ExternalInput")
    dr["s5"] = nc.dram_tensor("s5", (16, 1), f32, kind="ExternalInput")
    dr["b5"] = nc.dram_tensor("b5", (16, 1), f32, kind="ExternalInput")
    dr["bhead"] = nc.dram_tensor("bhead", (10, 1), f32, kind="ExternalInput")
    dr["id10"] = nc.dram_tensor("id10", (10, 10), f32, kind="ExternalInput")
    out = nc.dram_tensor("out", (NPER, 10), f32, kind="ExternalOutput")

    with tile.TileContext(nc) as tc:
        stat = tc.alloc_tile_pool(name="stat", bufs=1)
        scr = tc.alloc_tile_pool(name="scr", bufs=4)
        ps = tc.alloc_tile_pool(name="ps", bufs=2, space="PSUM")
        ps_pm = tc.alloc_tile_pool(name="ps_pm", bufs=1, space="PSUM")
        ps_ph = tc.alloc_tile_pool(name="ps_ph", bufs=1, space="PSUM")
        ps_tr = tc.alloc_tile_pool(name="ps_tr", bufs=2, space="PSUM")

        # --- static buffers ---
        xb = stat.tile([128, 30 * T], bf16, tag="xb")
        a2 = stat.tile([128, 30 * T], bf16, tag="a2")
        a3 = stat.tile([128, 16 * T], bf16, tag="a3")
        a4 = stat.tile([128, 16 * T], bf16, tag="a4")
        a5b = stat.tile([128, 9 * T], bf16, tag="a5b")
        y5 = stat.tile([128, 7 * T], bf16, tag="y5")

        wt = {}
        for l in range(1, 6):
            geo = LAYERS[l]
            K = (geo["W"] + 2) * geo["ci"]
            M = _mwidth(geo["W"], geo["co"], l in (2, 4))
            for ky in range(3):
                t = stat.tile([128, 128], bf16, tag=f"w{l}_{ky}")
                if l == 1:
                    for b_ in range(4):
                        nc.sync.dma_start(t[32 * b_:32 * b_ + K, 0:M],
                                          dr[f"wk{l}"].ap()[ky, :, :])
                elif l == 3:
                    for b_ in range(2):
                        nc.sync.dma_start(t[64 * b_:64 * b_ + K, 0:M],
                                          dr[f"wk{l}"].ap()[ky, :, :])
                else:
                    nc.sync.dma_start(t[0:K, 0:M], dr[f"wk{l}"].ap()[ky, :, :])
                wt[(l, ky)] = t
        Ssb = stat.tile([128, 16], bf16, tag="Ssb")
        nc.sync.dma_start(Ssb[0:112, 0:16], dr["Ssum"].ap())
        whd = stat.tile([16, 16], bf16, tag="whd")
        nc.sync.dma_start(whd[0:16, 0:10], dr["whead"].ap())
        id10 = stat.tile([10, 16], f32, tag="id10")
        nc.sync.dma_start(id10[0:10, 0:10], dr["id10"].ap())
        cvec = {}
        for name, P in [("thr1", 112), ("thr2", 120), ("thr3", 112),
                        ("thr4", 120), ("s5", 16), ("b5", 16), ("bhead", 10)]:
            t = stat.tile([128, 1], f32, tag="c_" + name)
            nc.sync.dma_start(t[0:P, 0:1], dr[name].ap())
            cvec[name] = t

        # --- init: zero x pads, set tilde-domain buffers (pads) to -1 ---
        nc.scalar.memzero(xb[:, :])
        nc.scalar.memzero(a4[:, 0:16 * T])
        for buf, fp in [(a2, 30 * T), (a3, 16 * T), (a5b, 9 * T)]:
            nc.scalar.memzero(buf[:, 0:fp])
            nc.vector.tensor_scalar_add(buf[:, 0:fp], buf[:, 0:fp], -1.0)

        def conv_rows(l, src, nrows, psum_for_row):
            """Emit 3 accumulated matmuls per output row; returns list of psum tiles."""
            geo = LAYERS[l]
            K = (geo["W"] + 2) * geo["ci"]
            M = geo["W"] * geo["co"]
            pts = []
            for y in range(nrows):
                pt = psum_for_row()
                for ky in range(3):
                    nc.tensor.matmul(
                        out=pt[0:M, :],
                        lhsT=wt[(l, ky)][0:K, 0:M],
                        rhs=src[0:K, (y + ky) * T:(y + ky + 1) * T],
                        start=(ky == 0),
                        stop=(ky == 2),
                        tile_position=(0, 0),
                    )
                pts.append(pt)
            return pts

        for blk in range(NBLK):
            ioff = blk * T
            # ---- load + sign x ----
            src = xT.ap()[:, ioff:ioff + T].rearrange("(r c) n -> c r n", r=28)
            for b_ in range(4):
                dst = xb[32 * b_:32 * b_ + 28, T:29 * T].rearrange(
                    "p (r t) -> p r t", r=28)
                nc.sync.dma_start(dst, src)

            # ---- L1 ----
            for yp in range(14):
                pt = ps.tile([128, 2 * T], f32, tag="pt")
                for half in range(2):
                    y = 2 * yp + half
                    b_ = 32 * (y % 4)
                    for ky in range(3):
                        nc.tensor.matmul(
                            out=pt[0:112, half * T:(half + 1) * T],
                            lhsT=wt[(1, ky)][b_:b_ + 30, 0:112],
                            rhs=xb[b_:b_ + 30, (y + ky) * T:(y + ky + 1) * T],
                            start=(ky == 0), stop=(ky == 2), tile_position=(b_, 0))
                nc.scalar.activation(
                    a2[0:112, (2 * yp + 1) * T:(2 * yp + 3) * T], pt[0:112, :],
                    ACT.Sign, bias=cvec["thr1"][0:112, 0:1])

            # ---- L2 (pool) ----
            for r in range(14):
                pt = ps.tile([128, 2 * T], f32, tag="pt")
                for half in range(2):
                    y = 2 * r + half
                    for ky in range(3):
                        nc.tensor.matmul(
                            out=pt[0:120, half * T:(half + 1) * T],
                            lhsT=wt[(2, ky)][0:120, 0:120],
                            rhs=a2[0:120, (y + ky) * T:(y + ky + 1) * T],
                            start=(ky == 0), stop=(ky == 2), tile_position=(0, 0))
                q0 = scr.tile([128, 2 * T], bf16, tag="q0")
                nc.scalar.activation(q0[0:120, :], pt[0:120, :], ACT.Sign,
                                     bias=cvec["thr2"][0:120, 0:1])
                sy = scr.tile([128, T], bf16, tag="sy")
                nc.vector.tensor_tensor(out=sy[0:120, :], in0=q0[0:120, 0:T],
                                        in1=q0[0:120, T:2 * T], op=ALU.max)
                sq = scr.tile([128, T], bf16, tag="sq")
                nc.vector.tensor_copy(sq[0:56, :], sy[64:120, :])
                nc.vector.tensor_tensor(out=a3[0:56, (r + 1) * T:(r + 2) * T],
                                        in0=sy[0:56, :], in1=sq[0:56, :],
                                        op=ALU.max)

            # ---- L3 (2-way row tiling; replicate a3 to the upper half) ----
            nc.vector.tensor_copy(a3[64:128, :], a3[0:64, :])
            for yp in range(7):
                pt = ps.tile([128, 2 * T], f32, tag="pt")
                for half in range(2):
                    y = 2 * yp + half
                    b_ = 64 * (y % 2)
                    for ky in range(3):
                        nc.tensor.matmul(
                            out=pt[0:112, half * T:(half + 1) * T],
                            lhsT=wt[(3, ky)][b_:b_ + 64, 0:112],
                            rhs=a3[b_:b_ + 64, (y + ky) * T:(y + ky + 1) * T],
                            start=(ky == 0), stop=(ky == 2), tile_position=(b_, 0))
                nc.vector.tensor_scalar(
                    out=a4[0:112, (2 * yp + 1) * T:(2 * yp + 3) * T],
                    in0=pt[0:112, :],
                    scalar1=cvec["thr3"][0:112, 0:1], scalar2=None,
                    op0=ALU.is_gt)

            # ---- L4 (pool) ----
            for r in range(7):
                pt = ps.tile([128, 2 * T], f32, tag="pt")
                for half in range(2):
                    y = 2 * r + half
                    for ky in range(3):
                        nc.tensor.matmul(
                            out=pt[0:120, half * T:(half + 1) * T],
                            lhsT=wt[(4, ky)][0:128, 0:120],
                            rhs=a4[0:128, (y + ky) * T:(y + ky + 1) * T],
                            start=(ky == 0), stop=(ky == 2), tile_position=(0, 0))
                q0 = scr.tile([128, 2 * T], bf16, tag="q0")
                nc.scalar.activation(q0[0:120, :], pt[0:120, :], ACT.Sign,
                                     bias=cvec["thr4"][0:120, 0:1])
                sy = scr.tile([128, T], bf16, tag="sy")
                nc.vector.tensor_tensor(out=sy[0:120, :], in0=q0[0:120, 0:T],
                                        in1=q0[0:120, T:2 * T], op=ALU.max)
                sq = scr.tile([128, T], bf16, tag="sq")
                nc.vector.tensor_copy(sq[0:56, :], sy[64:120, :])
                nc.vector.tensor_tensor(out=a5b[0:56, (r + 1) * T:(r + 2) * T],
                                        in0=sy[0:56, :], in1=sq[0:56, :],
                                        op=ALU.max)

            # ---- L5 conv -> y5 (raw values, bf16-exact halves) ----
            for y in range(7):
                pt = ps.tile([128, T], f32, tag="pt")
                for ky in range(3):
                    nc.tensor.matmul(
                        out=pt[0:112, :], lhsT=wt[(5, ky)][0:72, 0:112],
                        rhs=a5b[0:72, (y + ky) * T:(y + ky + 1) * T],
                        start=(ky == 0), stop=(ky == 2), tile_position=(0, 0))
                nc.vector.tensor_copy(y5[0:112, y * T:(y + 1) * T],
                                      pt[0:112, :])

            # ---- head: mean over 7x7 via 7 accumulated matmuls ----
            pm = ps_pm.tile([128, T], f32, tag="pm")
            for r in range(7):
                nc.tensor.matmul(
                    out=pm[0:16, :], lhsT=Ssb[0:112, 0:16],
                    rhs=y5[0:112, r * T:(r + 1) * T],
                    start=(r == 0), stop=(r == 6), tile_position=(0, 0))
            u = scr.tile([16, T], bf16, tag="u")
            nc.scalar.activation(u[0:16, :], pm[0:16, :], ACT.Sign,
                                 bias=cvec["b5"][0:16, 0:1],
                                 scale=cvec["s5"][0:16, 0:1])
            ph = ps_ph.tile([128, T], f32, tag="ph")
            nc.tensor.matmul(out=ph[0:10, :], lhsT=whd[0:16, 0:10],
                             rhs=u[0:16, :], start=True, stop=True,
                             tile_position=(0, 0))
            hh = scr.tile([16, T], f32, tag="hh")
            nc.scalar.activation(hh[0:10, :], ph[0:10, :], ACT.Identity,
                                 bias=cvec["bhead"][0:10, 0:1])

            for k in range(T // 128):
                ptr = ps_tr.tile([128, 16], f32, tag="ptr")
                nc.tensor.transpose(ptr[0:128, 0:10],
                                    hh[0:10, k * 128:(k + 1) * 128],
                                    id10[0:10, 0:10])
                mx = scr.tile([128, 1], f32, tag="mx")
                nc.vector.tensor_reduce(mx[0:128, 0:1], ptr[0:128, 0:10],
                                        axis=AX.X, op=ALU.max, negate=True)
                ex = scr.tile([128, 16], f32, tag="ex")
                nc.scalar.activation(ex[0:128, 0:10], ptr[0:128, 0:10],
                                     ACT.Exp, bias=mx[0:128, 0:1])
                sm = scr.tile([128, 1], f32, tag="sm")
                nc.vector.tensor_reduce(sm[0:128, 0:1], ex[0:128, 0:10],
                                        axis=AX.X, op=ALU.add)
                lg = scr.tile([128, 1], f32, tag="lg")
                nc.scalar.activation(lg[0:128, 0:1], sm[0:128, 0:1], ACT.Ln)
                tt = scr.tile([128, 1], f32, tag="tt")
                nc.vector.tensor_tensor(out=tt[0:128, 0:1], in0=mx[0:128, 0:1],
                                        in1=lg[0:128, 0:1], op=ALU.subtract)
                osb = scr.tile([128, 16], f32, tag="osb")
                nc.scalar.activation(osb[0:128, 0:10], ptr[0:128, 0:10],
                                     ACT.Identity, bias=tt[0:128, 0:1])
                row0 = ioff + k * 128
                nc.sync.dma_start(out.ap()[row0:row0 + 128, 0:10],
                                  osb[0:128, 0:10])

        for p in (ps_tr, ps_ph, ps_pm, ps, scr, stat):
            p.release()

    nc.compile()
    return nc


def kernel(**inputs):
    from concourse.bass_utils import run_bass_kernel_spmd
    import os

    if "nc" not in _CACHE:
        _CACHE["nc"] = _build()
    nc = _CACHE["nc"]

    folded = _host_fold(inputs)
    x = np.asarray(inputs["x"], np.float32).reshape(8192, 784)
    xT_full = np.ascontiguousarray(np.sign(x).T.astype(BF16))  # [784, 8192]

    in_maps = []
    for i in range(NCORE):
        m = {"xT": np.ascontiguousarray(xT_full[:, i * NPER:(i + 1) * NPER])}
        for k, v in folded.items():
            m[k] = v
        in_maps.append(m)

    res = run_bass_kernel_spmd(nc, in_maps, core_ids=list(range(NCORE)))
    _CACHE["last_result"] = res
    outs = [res.results[i]["out"] for i in range(NCORE)]
    return np.concatenate(outs, axis=0).astype(np.float32)


# revision 20
# speedup vs baseline: 1.3347x; 1.0494x over previous
"""Bass/Trainium2 kernel for binarized AlexNet-OWT-BN (MNIST-shaped), 8-core data parallel.

Contract: kernel(**inputs) takes the FULL unsharded inputs (x: [8192,1,28,28] f32
plus conv/bn/linear params) and returns the FULL [8192,10] f32 log-softmax output.

Design notes
------------
- Data parallel: batch 8192 -> 8 cores x 1024 images; each core runs 2 blocks of
  T=512 images (512 = fp32 PSUM bank limit for the matmul moving dim).
- The host ships sign(x) as bf16 (input staging, like the binarized weights) and
  folds conv bias + BN + ReLU + binarize of every layer into per-channel
  threshold tests applied directly to the conv PSUM: layers 1/2/4 via one
  ScalarE Sign (+-1 "tilde" activation domain, halved next-layer weights,
  0.5*sum(w) folded into the threshold, pad slots = -1), layer 3 via a DVE
  tensor_scalar is_gt ({0,1} domain, unhalved L4 weights, pad slots = 0) --
  splitting threshold work across both engines to balance them.
- BN channels with negative scale are handled by negating that channel's
  weights and threshold on the host, so max-pooling is always correct.
- Conv as matmul: partitions hold one padded image row (padded_ix x ci <= 128),
  M = out_width x co <= 128 (block-Toeplitz weights built on host), free dim =
  images; the 3 ky taps are PSUM-accumulated matmuls whose rhs differs only by
  a free-dim offset (rows live in the free dim: (padded_row, image)).
- Output rows are processed in pairs into one 2-bank PSUM tile [128, 1024] so
  each threshold/evacuation op covers 1024 free elements, halving the per-op
  PSUM-read overhead (172 cycles) on the bottleneck engines.
- 2x2 maxpool runs on the binarized SBUF values (PSUM allows only one
  tensor_tensor operand, and bf16 ops get the 2x DVE mode): y-pairs max
  lane-aligned, x-pairs use a parity-major M layout (even px at partitions
  0-55, odd px at 64-119) so one quadrant-aligned copy + one aligned max.
- conv1/conv3 use 4x/2x TensorE row tiling (tile_position row bands; conv1
  input bands come straight from a 4-way DMA fan-out, conv3's from one DVE
  copy) since K=30/64 << 128.
- Head: 7 accumulated matmuls against a channel-summing matrix implement the
  7x7 mean; Sign gives the binarized features; a 16->10 matmul, TensorE
  transpose per 128-image chunk, and a free-dim log-softmax finish.
"""

import sys

sys.path.insert(0, "/opt/trn_rl_repo")

import numpy as np
import ml_dtypes

EPS = 1e-5
T = 512
NBLK = 2
NCORE = 8
NPER = T * NBLK  # images per core

BF16 = ml_dtypes.bfloat16

# layer geometry: (ci, co, W_out, pooled)
LAYERS = {
    1: dict(ci=1, co=4, W=28),
    2: dict(ci=4, co=4, W=28),
    3: dict(ci=4, co=8, W=14),
    4: dict(ci=8, co=8, W=14),
    5: dict(ci=8, co=16, W=7),
}


def _krow(ix, c, W, ci):
    """K-partition index for padded column ix: real px first, pads at the end."""
    if ix == 0:
        return W * ci + c
    if ix == W + 1:
        return W * ci + ci + c
    return (ix - 1) * ci + c


def _mcol(ox, c, W, co, parity):
    """M column for out px ox; parity-major (evens/gap/odds) for pool layers."""
    if not parity:
        return ox * co + c
    half = (W // 2) * co
    pad_half = ((half + 31) // 32) * 32  # odds start at the next quadrant
    if ox % 2 == 0:
        return (ox // 2) * co + c
    return pad_half + (ox // 2) * co + c


def _mwidth(W, co, parity):
    if not parity:
        return W * co
    half = (W // 2) * co
    pad_half = ((half + 31) // 32) * 32
    return pad_half + half


def _toeplitz(wmat, W, parity):
    co, ci = wmat.shape[0], wmat.shape[1]
    K = (W + 2) * ci
    M = _mwidth(W, co, parity)
    out = []
    for ky in range(3):
        Wk = np.zeros((K, M), np.float32)
        for ox in range(W):
            for kx in range(3):
                ix = ox + kx
                for c_o in range(co):
                    for c_i in range(ci):
                        Wk[_krow(ix, c_i, W, ci),
                           _mcol(ox, c_o, W, co, parity)] = wmat[c_o, c_i, ky, kx]
        out.append(Wk)
    return np.stack(out)  # [3, K, M]


def _host_fold(inputs):
    """Fold weights + BN into Toeplitz matmul weights and threshold vectors."""
    d = {}
    for l in range(1, 6):
        tag = str(l)
        w = np.asarray(inputs["w" + tag], np.float64)
        b = np.asarray(inputs["b" + tag], np.float64)
        g = np.asarray(inputs["g" + tag], np.float64)
        be = np.asarray(inputs["be" + tag], np.float64)
        m = np.asarray(inputs["m" + tag], np.float64)
        v = np.asarray(inputs["v" + tag], np.float64)
        wb = np.sign(w).astype(np.float64)
        s = g / np.sqrt(v + EPS)
        geo = LAYERS[l]
        co, W = geo["co"], geo["W"]
        if l <= 4:
            # threshold: bn(y_conv) > 0  <=>  psum + bias > 0 (after folding)
            c = (b - m) + be / s
            flip = np.where(s < 0, -1.0, 1.0)
            wb = wb * flip[:, None, None, None]
            c = c * flip
            if l == 1:
                wmat, kap = wb, np.zeros(co)
            elif l == 4:
                # L3's output comes from a DVE is_gt -> {0,1} domain
                wmat, kap = wb, np.zeros(co)
            else:
                wmat = wb * 0.5
                kap = 0.5 * wb.sum(axis=(1, 2, 3))
            parity = l in (2, 4)
            bias_ch = (kap + c).astype(np.float32)  # per channel, > 0 test
            M = _mwidth(W, co, parity)
            bias_vec = np.zeros((M, 1), np.float32)
            for ox in range(W):
                for c_o in range(co):
                    bias_vec[_mcol(ox, c_o, W, co, parity), 0] = bias_ch[c_o]
            if l == 3:
                bias_vec = -bias_vec  # device uses is_gt(psum, -bias)
            d[f"thr{l}"] = bias_vec
            d[f"wk{l}"] = _toeplitz(wmat.astype(np.float32), W, parity).astype(BF16)
        else:
            wmat = wb * 0.5
            kap = 0.5 * wb.sum(axis=(1, 2, 3))
            d["wk5"] = _toeplitz(wmat.astype(np.float32), W, False).astype(BF16)
            # head mean+bn5: z = (s5/49)*psum_sum + s5*(kap+b5-m5)+be5
            d["s5"] = (s / 49.0).astype(np.float32).reshape(-1, 1)
            d["b5"] = (s * (kap + b - m) + be).astype(np.float32).reshape(-1, 1)
    # channel-summing matrix for the 7x7 mean: [112=(ox,co), 16]
    S = np.zeros((7 * 16, 16), np.float32)
    for ox in range(7):
        for c_ in range(16):
            S[ox * 16 + c_, c_] = 1.0
    d["Ssum"] = S.astype(BF16)
    wl = np.sign(np.asarray(inputs["wl"], np.float64))  # [10, 16]
    bl = np.asarray(inputs["bl"], np.float64)
    d["whead"] = (wl.T * 0.5).astype(BF16)  # [16, 10] lhsT
    d["bhead"] = (bl + 0.5 * wl.sum(axis=1)).astype(np.float32).reshape(-1, 1)
    d["id10"] = np.eye(10, dtype=np.float32)
    return d


_CACHE = {}


def _build():
    from concourse import bacc, tile, mybir

    f32 = mybir.dt.float32
    bf16 = mybir.dt.bfloat16
    ACT = mybir.ActivationFunctionType
    ALU = mybir.AluOpType
    AX = mybir.AxisListType

    nc = bacc.Bacc("TRN2", num_devices=NCORE)

    xT = nc.dram_tensor("xT", (784, NPER), bf16, kind="ExternalInput")
    dr = {}
    for l in range(1, 6):
        geo = LAYERS[l]
        K = (geo["W"] + 2) * geo["ci"]
        M = _mwidth(geo["W"], geo["co"], l in (2, 4))
        dr[f"wk{l}"] = nc.dram_tensor(f"wk{l}", (3, K, M), bf16, kind="ExternalInput")
        if l <= 4:
            dr[f"thr{l}"] = nc.dram_tensor(f"thr{l}", (M, 1), f32, kind="ExternalInput")
    dr["Ssum"] = nc.dram_tensor("Ssum", (112, 16), bf16, kind="ExternalInput")
    dr["whead"] = nc.dram_tensor("whead", (16, 10), bf16, kind="ExternalInput")
    dr["s5"] = nc.dram_tensor("s5", (16, 1), f32, kind="ExternalInput")
    dr["b5"] = nc.dram_tensor("b5", (16, 1), f32, kind="ExternalInput")
    dr["bhead"] = nc.dram_tensor("bhead", (10, 1), f32, kind="ExternalInput")
    dr["id10"] = nc.dram_tensor("id10", (10, 10), f32, kind="ExternalInput")
    out = nc.dram_tensor("out", (NPER, 10), f32, kind="ExternalOutput")

    with tile.TileContext(nc) as tc:
        stat = tc.alloc_tile_pool(name="stat", bufs=1)
        scr = tc.alloc_tile_pool(name="scr", bufs=4)
        ps = tc.alloc_tile_pool(name="ps", bufs=2, space="PSUM")
        ps_pm = tc.alloc_tile_pool(name="ps_pm", bufs=1, space="PSUM")
        ps_ph = tc.alloc_tile_pool(name="ps_ph", bufs=1, space="PSUM")
        ps_tr = tc.alloc_tile_pool(name="ps_tr", bufs=2, space="PSUM")

        # --- static buffers ---
        xb = stat.tile([128, 30 * T], bf16, tag="xb")
        a2 = stat.tile([128, 30 * T], bf16, tag="a2")
        a3 = stat.tile([128, 16 * T], bf16, tag="a3")
        a4 = stat.tile([128, 16 * T], bf16, tag="a4")
        a5b = stat.tile([128, 9 * T], bf16, tag="a5b")
        y5 = stat.tile([128, 7 * T], bf16, tag="y5")

        wt = {}
        for l in range(1, 6):
            geo = LAYERS[l]
            K = (geo["W"] + 2) * geo["ci"]
            M = _mwidth(geo["W"], geo["co"], l in (2, 4))
            for ky in range(3):
                t = stat.tile([128, 128], bf16, tag=f"w{l}_{ky}")
                if l == 1:
                    for b_ in range(4):
                        nc.sync.dma_start(t[32 * b_:32 * b_ + K, 0:M],
                                          dr[f"wk{l}"].ap()[ky, :, :])
                elif l == 3:
                    for b_ in range(2):
                        nc.sync.dma_start(t[64 * b_:64 * b_ + K, 0:M],
                                          dr[f"wk{l}"].ap()[ky, :, :])
                else:
                    nc.sync.dma_start(t[0:K, 0:M], dr[f"wk{l}"].ap()[ky, :, :])
                wt[(l, ky)] = t
        Ssb = stat.tile([128, 16], bf16, tag="Ssb")
        nc.sync.dma_start(Ssb[0:112, 0:16], dr["Ssum"].ap())
        whd = stat.tile([16, 16], bf16, tag="whd")
        nc.sync.dma_start(whd[0:16, 0:10], dr["whead"].ap())
        id10 = stat.tile([10, 16], f32, tag="id10")
        nc.sync.dma_start(id10[0:10, 0:10], dr["id10"].ap())
        cvec = {}
        for name, P in [("thr1", 112), ("thr2", 120), ("thr3", 112),
                        ("thr4", 120), ("s5", 16), ("b5", 16), ("bhead", 10)]:
            t = stat.tile([128, 1], f32, tag="c_" + name)
            nc.sync.dma_start(t[0:P, 0:1], dr[name].ap())
            cvec[name] = t

        # --- init: zero x pads, set tilde-domain buffers (pads) to -1 ---
        nc.scalar.memzero(xb[:, :])
        nc.scalar.memzero(a4[:, 0:16 * T])
        for buf, fp in [(a2, 30 * T), (a3, 16 * T), (a5b, 9 * T)]:
            nc.scalar.memzero(buf[:, 0:fp])
            nc.vector.tensor_scalar_add(buf[:, 0:fp], buf[:, 0:fp], -1.0)

        def conv_rows(l, src, nrows, psum_for_row):
            """Emit 3 accumulated matmuls per output row; returns list of psum tiles."""
            geo = LAYERS[l]
            K = (geo["W"] + 2) * geo["ci"]
            M = geo["W"] * geo["co"]
            pts = []
            for y in range(nrows):
                pt = psum_for_row()
                for ky in range(3):
                    nc.tensor.matmul(
                        out=pt[0:M, :],
                        lhsT=wt[(l, ky)][0:K, 0:M],
                        rhs=src[0:K, (y + ky) * T:(y + ky + 1) * T],
                        start=(ky == 0),
                        stop=(ky == 2),
                        tile_position=(0, 0),
                    )
                pts.append(pt)
            return pts

        for blk in range(NBLK):
            ioff = blk * T
            # ---- load + sign x ----
            src = xT.ap()[:, ioff:ioff + T].rearrange("(r c) n -> c r n", r=28)
            for b_ in range(4):
                dst = xb[32 * b_:32 * b_ + 28, T:29 * T].rearrange(
                    "p (r t) -> p r t", r=28)
                nc.sync.dma_start(dst, src)

            # ---- L1 ----
            for yp in range(14):
                pt = ps.tile([128, 2 * T], f32, tag="pt")
                for half in range(2):
                    y = 2 * yp + half
                    b_ = 32 * (y % 4)
                    for ky in range(3):
                        nc.tensor.matmul(
                            out=pt[0:112, half * T:(half + 1) * T],
                            lhsT=wt[(1, ky)][b_:b_ + 30, 0:112],
                            rhs=xb[b_:b_ + 30, (y + ky) * T:(y + ky + 1) * T],
                            start=(ky == 0), stop=(ky == 2), tile_position=(b_, 0))
                nc.scalar.activation(
                    a2[0:112, (2 * yp + 1) * T:(2 * yp + 3) * T], pt[0:112, :],
                    ACT.Sign, bias=cvec["thr1"][0:112, 0:1])

            # ---- L2 (pool) ----
            for r in range(14):
                pt = ps.tile([128, 2 * T], f32, tag="pt")
                for half in range(2):
                    y = 2 * r + half
                    for ky in range(3):
                        nc.tensor.matmul(
                            out=pt[0:120, half * T:(half + 1) * T],
                            lhsT=wt[(2, ky)][0:120, 0:120],
                            rhs=a2[0:120, (y + ky) * T:(y + ky + 1) * T],
                            start=(ky == 0), stop=(ky == 2), tile_position=(0, 0))
                q0 = scr.tile([128, 2 * T], bf16, tag="q0")
                nc.scalar.activation(q0[0:120, :], pt[0:120, :], ACT.Sign,
                                     bias=cvec["thr2"][0:120, 0:1])
                sy = scr.tile([128, T], bf16, tag="sy")
                nc.vector.tensor_tensor(out=sy[0:120, :], in0=q0[0:120, 0:T],
                                        in1=q0[0:120, T:2 * T], op=ALU.max)
                sq = scr.tile([128, T], bf16, tag="sq")
                nc.vector.tensor_copy(sq[0:56, :], sy[64:120, :])
                nc.vector.tensor_tensor(out=a3[0:56, (r + 1) * T:(r + 2) * T],
                                        in0=sy[0:56, :], in1=sq[0:56, :],
                                        op=ALU.max)

            # ---- L3 (2-way row tiling; replicate a3 to the upper half) ----
            nc.vector.tensor_copy(a3[64:128, :], a3[0:64, :])
            for yp in range(7):
                pt = ps.tile([128, 2 * T], f32, tag="pt")
                for half in range(2):
                    y = 2 * yp + half
                    b_ = 64 * (y % 2)
                    for ky in range(3):
                        nc.tensor.matmul(
                            out=pt[0:112, half * T:(half + 1) * T],
                            lhsT=wt[(3, ky)][b_:b_ + 64, 0:112],
                            rhs=a3[b_:b_ + 64, (y + ky) * T:(y + ky + 1) * T],
                            start=(ky == 0), stop=(ky == 2), tile_position=(b_, 0))
                nc.vector.tensor_scalar(
                    out=a4[0:112, (2 * yp + 1) * T:(2 * yp + 3) * T],
                    in0=pt[0:112, :],
                    scalar1=cvec["thr3"][0:112, 0:1], scalar2=None,
                    op0=ALU.is_gt)

            # ---- L4 (pool) ----
            for r in range(7):
                pt = ps.tile([128, 2 * T], f32, tag="pt")
                for half in range(2):
                    y = 2 * r + half
                    for ky in range(3):
                        nc.tensor.matmul(
                            out=pt[0:120, half * T:(half + 1) * T],
                            lhsT=wt[(4, ky)][0:128, 0:120],
                            rhs=a4[0:128, (y + ky) * T:(y + ky + 1) * T],
                            start=(ky == 0), stop=(ky == 2), tile_position=(0, 0))
                q0 = scr.tile([128, 2 * T], bf16, tag="q0")
                nc.scalar.activation(q0[0:120, :], pt[0:120, :], ACT.Sign,
                                     bias=cvec["thr4"][0:120, 0:1])
                sy = scr.tile([128, T], bf16, tag="sy")
                nc.vector.tensor_tensor(out=sy[0:120, :], in0=q0[0:120, 0:T],
                                        in1=q0[0:120, T:2 * T], op=ALU.max)
                sq = scr.tile([128, T], bf16, tag="sq")
                nc.vector.tensor_copy(sq[0:56, :], sy[64:120, :])
                nc.vector.tensor_tensor(out=a5b[0:56, (r + 1) * T:(r + 2) * T],
                                        in0=sy[0:56, :], in1=sq[0:56, :],
                                        op=ALU.max)

            # ---- L5 conv -> y5 (raw values, bf16-exact halves) ----
            for y in range(7):
                pt = ps.tile([128, T], f32, tag="pt")
                for ky in range(3):
                    nc.tensor.matmul(
                        out=pt[0:112, :], lhsT=wt[(5, ky)][0:72, 0:112],
                        rhs=a5b[0:72, (y + ky) * T:(y + ky + 1) * T],
                        start=(ky == 0), stop=(ky == 2), tile_position=(0, 0))
                nc.vector.tensor_copy(y5[0:112, y * T:(y + 1) * T],
                                      pt[0:112, :])

            # ---- head: mean over 7x7 via 7 accumulated matmuls ----
            pm = ps_pm.tile([128, T], f32, tag="pm")
            for r in range(7):
                nc.tensor.matmul(
                    out=pm[0:16, :], lhsT=Ssb[0:112, 0:16],
                    rhs=y5[0:112, r * T:(r + 1) * T],
                    start=(r == 0), stop=(r == 6), tile_position=(0, 0))
            u = scr.tile([16, T], bf16, tag="u")
            nc.scalar.activation(u[0:16, :], pm[0:16, :], ACT.Sign,
                                 bias=cvec["b5"][0:16, 0:1],
                                 scale=cvec["s5"][0:16, 0:1])
            ph = ps_ph.tile([128, T], f32, tag="ph")
            nc.tensor.matmul(out=ph[0:10, :], lhsT=whd[0:16, 0:10],
                             rhs=u[0:16, :], start=True, stop=True,
                             tile_position=(0, 0))
            hh = scr.tile([16, T], f32, tag="hh")
            nc.scalar.activation(hh[0:10, :], ph[0:10, :], ACT.Identity,
                                 bias=cvec["bhead"][0:10, 0:1])

            for k in range(T // 128):
                ptr = ps_tr.tile([128, 16], f32, tag="ptr")
                nc.tensor.transpose(ptr[0:128, 0:10],
                                    hh[0:10, k * 128:(k + 1) * 128],
                                    id10[0:10, 0:10])
                mx = scr.tile([128, 1], f32, tag="mx")
                nc.vector.tensor_reduce(mx[0:128, 0:1], ptr[0:128, 0:10],
                                        axis=AX.X, op=ALU.max, negate=True)
                ex = scr.tile([128, 16], f32, tag="ex")
                nc.scalar.activation(ex[0:128, 0:10], ptr[0:128, 0:10],
                                     ACT.Exp, bias=mx[0:128, 0:1])
                sm = scr.tile([128, 1], f32, tag="sm")
                nc.vector.tensor_reduce(sm[0:128, 0:1], ex[0:128, 0:10],
                                        axis=AX.X, op=ALU.add)
                lg = scr.tile([128, 1], f32, tag="lg")
                nc.scalar.activation(lg[0:128, 0:1], sm[0:128, 0:1], ACT.Ln)
                tt = scr.tile([128, 1], f32, tag="tt")
                nc.vector.tensor_tensor(out=tt[0:128, 0:1], in0=mx[0:128, 0:1],
                                        in1=lg[0:128, 0:1], op=ALU.subtract)
                osb = scr.tile([128, 16], f32, tag="osb")
                nc.scalar.activation(osb[0:128, 0:10], ptr[0:128, 0:10],
                                     ACT.Identity, bias=tt[0:128, 0:1])
                row0 = ioff + k * 128
                nc.sync.dma_start(out.ap()[row0:row0 + 128, 0:10],
                                  osb[0:128, 0:10])

        for p in (ps_tr, ps_ph, ps_pm, ps, scr, stat):
            p.release()

    nc.compile()
    return nc


def kernel(**inputs):
    from concourse.bass_utils import run_bass_kernel_spmd
    import os

    if "nc" not in _CACHE:
        _CACHE["nc"] = _build()
    nc = _CACHE["nc"]

    folded = _host_fold(inputs)
    x = np.asarray(inputs["x"], np.float32).reshape(8192, 784)
    xT_full = np.ascontiguousarray(np.sign(x).T.astype(BF16))  # [784, 8192]

    in_maps = []
    for i in range(NCORE):
        m = {"xT": np.ascontiguousarray(xT_full[:, i * NPER:(i + 1) * NPER])}
        for k, v in folded.items():
            m[k] = v
        in_maps.append(m)

    res = run_bass_kernel_spmd(nc, in_maps, core_ids=list(range(NCORE)))
    _CACHE["last_result"] = res
    outs = [res.results[i]["out"] for i in range(NCORE)]
    return np.concatenate(outs, axis=0).astype(np.float32)
